# revision 26
# baseline (speedup 1.0000x reference)
"""Trainium2 Bass kernel for nn_EpiNN_aaindex (pairwise-MLP GNN reduction).

Math (per batch b):
  x1 = emb@tw + tb
  X[i,d] = emb[i*64+d] * tw[i*64+d]            (L=256, D=64)
  s_ij = MLP(concat[(x_i+x_j)/2, |x_i-x_j|])   (64->16->1, LeakyReLU 0.01)
  out_b = x1 + scale * sum_{i<j} s_ij

Strategy: 8 cores, 4 batches/core (data parallel over B=32).
Exact upper-triangle enumeration via cyclic offsets o=1..128:
pairs (i, (i+o) mod 256) for o=1..127 cover each unordered pair once;
o=128 covers each of its 128 pairs twice (weighted 0.5 on the host).

Layouts (per batch, SBUF, bf16):
  XU  [128, 512]: both partition halves = [X2T | X2T]  (X2T = X.T [64, 256])
  XSo [128, 512]: top = X2T shifted 1, bottom = shifted 65   (odd offsets)
  XSe [128, 512]: top = X2T shifted 2, bottom = shifted 66   (even offsets)
Unit u = offset pair (u, u+64); dual-unit d = units (2d+1, 2d+2).
A2 [128, 512] = |x_i - x_j| features for 4 offsets (2 per partition half).
P1 [128, 512] psum quadrants = pre1 for the 4 offsets (3 matmuls each:
  w1b@A, 0.5*w1a@X2T (u_i term), 0.5*w1a@X2T-shifted (u_j term)).
ACT Lrelu (+b1 bias) -> H1 bf16 -> 4 layer-2 matmuls -> P2 -> ACT Lrelu
(+b2) with accum_out giving the free-dim (i) sums for free.

Final combine on host: out = x1 + scale*(w3 . R + 32640*b3).
"""
import numpy as np

L, D = 256, 64
B_PER_CORE = 4
N_CORES = 8
NPAIRS = 32640  # 256*255/2

_CACHE = {}
import os as _os
N_DUALS = int(_os.environ.get("EPINN_DUALS", "32"))
N_BATCH = int(_os.environ.get("EPINN_BATCH", str(B_PER_CORE)))
N_RUN_CORES = int(_os.environ.get("EPINN_CORES", str(N_CORES)))
STAGE = int(_os.environ.get("EPINN_STAGE", "9"))
VAR = _os.environ.get("EPINN_VAR", "")


def _build_program():
    import concourse.bacc as bacc
    import concourse.bass as bass
    import concourse.mybir as mybir
    import concourse.tile as tile
    from contextlib import ExitStack

    f32 = mybir.dt.float32
    bf16 = mybir.dt.bfloat16
    u16 = mybir.dt.uint16
    AF = mybir.ActivationFunctionType
    ALU = mybir.AluOpType
    AF_MAIN = AF.Relu if _os.environ.get("EPINN_RELU") else AF.Lrelu

    nc = bacc.Bacc("TRN2", target_bir_lowering=False, debug=False,
                   num_devices=N_CORES)

    # ---- DRAM parameters (per core) ----
    emb_d = nc.declare_dram_parameter("emb4", [B_PER_CORE, L * D + 1], f32,
                                      isOutput=False)
    w1bt_d = nc.declare_dram_parameter("w1bt", [64, 64], bf16, isOutput=False)
    w1at_d = nc.declare_dram_parameter("w1at", [64, 64], bf16, isOutput=False)
    w2p8_d = nc.declare_dram_parameter("w2p8", [128, 128], bf16, isOutput=False)
    b1s_d = nc.declare_dram_parameter("b1s", [128, 1], f32, isOutput=False)
    b2s8_d = nc.declare_dram_parameter("b2s8", [128, 1], f32, isOutput=False)
    twp_d = nc.declare_dram_parameter("twp", [L, D], f32, isOutput=False)
    twl_d = nc.declare_dram_parameter("twl", [1, 1], f32, isOutput=False)

    acc_o = nc.declare_dram_parameter("acc_o", [B_PER_CORE, 128, 9], f32,
                                      isOutput=True)
    x1_o = nc.declare_dram_parameter("x1_o", [B_PER_CORE, 1, 1], f32,
                                     isOutput=True)

    with tile.TileContext(nc) as tc, ExitStack() as ctx:
        cpool = ctx.enter_context(tc.tile_pool(name="consts", bufs=1))
        xpool = ctx.enter_context(tc.tile_pool(name="xbufs", bufs=2))
        apool = ctx.enter_context(tc.tile_pool(name="abufs", bufs=3))
        hpool = ctx.enter_context(tc.tile_pool(name="hbufs", bufs=3))
        jpool = ctx.enter_context(tc.tile_pool(name="junk", bufs=2))
        opool = ctx.enter_context(tc.tile_pool(name="outs", bufs=2))
        pp1 = ctx.enter_context(tc.tile_pool(name="p1", bufs=2, space="PSUM"))
        pp2 = ctx.enter_context(tc.tile_pool(name="p2", bufs=2, space="PSUM"))
        ppt = ctx.enter_context(tc.tile_pool(name="pt", bufs=1, space="PSUM"))
        ppx = ctx.enter_context(tc.tile_pool(name="px", bufs=1, space="PSUM"))

        # ---- static weights / consts ----
        WB = cpool.tile([128, 64], bf16)
        WA = cpool.tile([128, 64], bf16)
        W2P8 = cpool.tile([128, 128], bf16)
        B1S = cpool.tile([128, 1], f32)
        B2S8 = cpool.tile([128, 1], f32)
        TWP = cpool.tile([128, 2, 64], f32)  # [128p, (half, d)]
        TWL = cpool.tile([1, 1], f32)
        IDENT = cpool.tile([128, 128], f32)
        ONES = cpool.tile([128, 1], f32)

        E2_0 = xpool.tile([128, 2, 64], f32, tag="e2")
        nc.sync.dma_start(
            E2_0[:], emb_d[0, 0:L * D].rearrange("(h p f) -> p h f", p=128, f=64)
        )
        nc.gpsimd.dma_start(TWP[:, 0, :], twp_d[0:128, :])
        nc.gpsimd.dma_start(TWP[:, 1, :], twp_d[128:256, :])
        nc.sync.dma_start(WB[0:64, :], w1bt_d[:])
        nc.sync.dma_start(WB[64:128, :], w1bt_d[:])
        nc.sync.dma_start(WA[0:64, :], w1at_d[:])
        nc.sync.dma_start(WA[64:128, :], w1at_d[:])
        nc.sync.dma_start(W2P8[:], w2p8_d[:])
        nc.scalar.dma_start(B1S[:], b1s_d[:])
        nc.scalar.dma_start(B2S8[:], b2s8_d[:])
        nc.scalar.dma_start(TWL[:], twl_d[:])
        nc.gpsimd.memset(IDENT[:], 0.0)
        nc.gpsimd.affine_select(
            out=IDENT[:], in_=IDENT[:], compare_op=ALU.not_equal, fill=1.0,
            base=0, pattern=[[-1, 128]], channel_multiplier=1,
        )
        nc.gpsimd.memset(ONES[:], 1.0)

        nit = N_DUALS // 2

        def emit_setup_a(b):
            """emb load, X = emb*tw, x1 scalar path, transpose to PT."""
            if b == 0:
                E2 = E2_0
            else:
                E2 = xpool.tile([128, 2, 64], f32, tag="e2")
                nc.sync.dma_start(
                    E2[:], emb_d[b, 0:L * D].rearrange("(h p f) -> p h f",
                                                       p=128, f=64)
                )
            EL = xpool.tile([1, 1], f32, tag="el")
            nc.sync.dma_start(EL[:], emb_d[b, L * D:L * D + 1][None, :])

            X2F = xpool.tile([128, 2, 64], f32, tag="x2f")
            nc.vector.tensor_tensor(out=X2F[:], in0=E2[:], in1=TWP[:],
                                    op=ALU.mult)

            # x1 = sum(X2F) + EL*twl + tb  (tb added on host)
            CS = xpool.tile([128, 2], f32, tag="cs")
            nc.vector.tensor_reduce(out=CS[:, 0:1], in_=X2F[:, 0, :],
                                    op=ALU.add, axis=mybir.AxisListType.X)
            nc.vector.tensor_reduce(out=CS[:, 1:2], in_=X2F[:, 1, :],
                                    op=ALU.add, axis=mybir.AxisListType.X)
            CS1 = xpool.tile([128, 1], f32, tag="cs1")
            nc.vector.tensor_tensor(out=CS1[:], in0=CS[:, 0:1], in1=CS[:, 1:2],
                                    op=ALU.add)
            PX1 = ppx.tile([1, 1], f32, tag="px1")
            nc.tensor.matmul(PX1[:], CS1[:], ONES[:], start=True, stop=False,
                             skip_group_check=True)
            nc.tensor.matmul(PX1[:], EL[:], TWL[:], start=False, stop=True,
                             skip_group_check=True)
            X1S = xpool.tile([1, 1], f32, tag="x1s")
            nc.vector.tensor_scalar(out=X1S[:], in0=PX1[:], scalar1=1.0,
                                    scalar2=None, op0=ALU.mult)
            nc.sync.dma_start(x1_o[b], X1S[:])

            # transpose X -> X2T [64d, 256i] in psum
            PT = ppt.tile([64, 256], f32, tag="pt")
            nc.tensor.matmul(PT[:, 0:128], X2F[:, 0, :], IDENT[:],
                             is_transpose=True, start=True, stop=True,
                             skip_group_check=True)
            nc.tensor.matmul(PT[:, 128:256], X2F[:, 1, :], IDENT[:],
                             is_transpose=True, start=True, stop=True,
                             skip_group_check=True)
            return PT

        def emit_setup_b(PT):
            """XU tile: [X2T | X2T] on both partition halves."""
            XU = xpool.tile([128, 512], bf16, tag="xu")
            nc.vector.tensor_scalar(out=XU[0:64, 0:256], in0=PT[:],
                                    scalar1=1.0, scalar2=None, op0=ALU.mult)
            nc.sync.dma_start(XU[0:64, 256:512], XU[0:64, 0:256])
            nc.sync.dma_start(XU[64:128, :], XU[0:64, :])
            return XU

        def emit_setup_c(XU):
            """XSo/XSe shifted tiles (spread across scalar/gpsimd queues)."""
            XSo = xpool.tile([128, 512], bf16, tag="xso")
            XSe = xpool.tile([128, 512], bf16, tag="xse")
            # XSo: top shift 1, bottom shift 65
            nc.scalar.dma_start(XSo[0:64, 0:511], XU[0:64, 1:512])
            nc.scalar.dma_start(XSo[0:64, 511:512], XU[0:64, 255:256])
            nc.scalar.dma_start(XSo[64:128, 0:447], XU[0:64, 65:512])
            nc.scalar.dma_start(XSo[64:128, 447:512], XU[0:64, 0:65])
            # XSe: top shift 2, bottom shift 66
            nc.gpsimd.dma_start(XSe[0:64, 0:510], XU[0:64, 2:512])
            nc.gpsimd.dma_start(XSe[0:64, 510:512], XU[0:64, 254:256])
            nc.gpsimd.dma_start(XSe[64:128, 0:446], XU[0:64, 66:512])
            nc.gpsimd.dma_start(XSe[64:128, 446:512], XU[0:64, 0:66])
            return XSo, XSe

        def build_a2(tiles, it):
            XU, XSo, XSe = tiles
            A2 = apool.tile([128, 1024], bf16, tag="a2")
            # cols 0:256   = shift (4it+1 | +65)   from XSo offset 4it
            # cols 256:512 = shift (4it+3 | +67)   from XSo offset 4it+2
            # cols 512:768 = shift (4it+2 | +66)   from XSe offset 4it
            # cols 768:1024= shift (4it+4 | +68)   from XSe offset 4it+2
            c0 = 4 * it
            in0b = XU[:, 0:256].unsqueeze(1).broadcast_to([128, 2, 256])
            for (dst, src) in ((A2[:, 0:512], XSo), (A2[:, 512:1024], XSe)):
                sl = src[:, c0:c0 + 258]
                in1w = bass.AP(tensor=sl.tensor, offset=sl.offset,
                               ap=[list(sl.ap[0]), [2, 2], [1, 256]])
                nc.vector.tensor_tensor(
                    out=dst.rearrange("p (a b) -> p a b", a=2),
                    in0=in0b, in1=in1w, op=ALU.subtract)
            nc.vector.tensor_scalar(
                out=A2[:].bitcast(u16), in0=A2[:].bitcast(u16),
                scalar1=0x7FFF, scalar2=None, op0=ALU.bitwise_and)
            return A2

        def emit_p1(tiles, A2, it):
            XU, XSo, XSe = tiles
            c0 = 4 * it
            # P1a: T0 (rows 0:64 <- o=u1,u1p) T10 (rows 64:128 <- +64)
            # P1b: T8 (rows 0:64 <- o=u2+64,u2p+64) T2 (rows 64:128 <- u2,u2p)
            P1 = pp1.tile([128, 1024], f32, tag="p1")
            for (pc, tp, ar) in (
                (0, (0, 0), 0),        # T0
                (0, (64, 64), 64),     # T10
                (512, (64, 0), 64),    # T8  (array rows 64-127 -> psum 0-63)
                (512, (0, 64), 0),     # T2  (array rows 0-63 -> psum 64-127)
            ):
                rg, pr = ar, tp[1]
                XSx = XSo if pc == 0 else XSe
                ps = P1[pr:pr + 64, pc:pc + 512]
                nc.tensor.matmul(ps, WB[rg:rg + 64, :], A2[rg:rg + 64, pc:pc + 512],
                                 start=True, stop=False, tile_position=tp,
                                 skip_group_check=True)
                nc.tensor.matmul(ps, WA[rg:rg + 64, :],
                                 XU[rg:rg + 64, 0:512],
                                 start=False, stop=False, tile_position=tp,
                                 skip_group_check=True)
                nc.tensor.matmul(ps[:, 0:256], WA[rg:rg + 64, :],
                                 XSx[rg:rg + 64, c0:c0 + 256],
                                 start=False, stop=False, tile_position=tp,
                                 skip_group_check=True)
                nc.tensor.matmul(ps[:, 256:512], WA[rg:rg + 64, :],
                                 XSx[rg:rg + 64, c0 + 2:c0 + 258],
                                 start=False, stop=True, tile_position=tp,
                                 skip_group_check=True)
            H1 = hpool.tile([128, 1024], bf16, tag="h1")
            nc.scalar.activation(H1[:], P1[:], AF_MAIN, bias=B1S[:],
                                 scale=1.0, alpha=0.01)
            return H1

        p2state = [None]

        def emit_layer2(ACC, H1, it):
            # layer 2: pack 8 offsets x 16 outputs across 128 psum
            # partitions; P2 free dim is only 256 -> 4x cheaper lrelu2.
            # One 128-contraction MM per 32-partition strip: weight rows
            # 0:64 (H1 top offset) land in cols 0:16, rows 64:128 in
            # 16:32 -- avoids two concurrent drains on one strip.
            # Two consecutive iterations share one 2-bank P2F tile (cols
            # 0:256 / 256:512) so lrelu2 + reduce run once per pair.
            # it=14 and the o=128 iteration (15) stay unpaired so the host
            # can halve the double-counted o=128 block (ACC cols 7 and 8).
            if it == 15:
                pair, po, col = (15,), 0, 8
            elif it == 14:
                pair, po, col = (14,), 0, 7
            else:
                pair, po, col = (it - it % 2, it - it % 2 + 1), 256 * (it % 2), it // 2
            if po == 0:
                P2F = pp2.tile([128, 512], f32, tag="p2")
                p2state[0] = P2F
            else:
                P2F = p2state[0]
            for cb in range(4):
                hc = 256 * cb
                nc.tensor.matmul(P2F[32 * cb:32 * cb + 32, po:po + 256],
                                 W2P8[:, 32 * cb:32 * cb + 32],
                                 H1[:, hc:hc + 256],
                                 start=True, stop=True,
                                 tile_position=(0, 32 * cb),
                                 skip_group_check=True)
            if it != pair[-1]:
                return
            n = 256 * len(pair)
            # lrelu2 on ACT (bias free), column-sum over i on DVE
            H2 = jpool.tile([128, 512], bf16, tag="h2")
            nc.scalar.activation(H2[:, 0:n], P2F[:, 0:n], AF_MAIN,
                                 bias=B2S8[:], scale=1.0, alpha=0.01)
            nc.vector.tensor_reduce(out=ACC[:, col:col + 1], in_=H2[:, 0:n],
                                    op=ALU.add, axis=mybir.AxisListType.X)

        # ---- main loops: 16 iterations x 8 offsets per batch ----
        # offsets at iteration it: T0 stream: u1=4it+1, u1p=4it+3
        #                          T2/T8 stream: u2=4it+2, u2p=4it+4
        #                          +64 variants on the bottom halves
        # Software-pipelined so the PE queue never stalls: A2 is built one
        # iteration ahead (DVE overlaps P1[it]); layer2[it-1] (whose MMs wait
        # on act1[it-1], finished during P1[it]) is emitted after act1[it];
        # the NEXT batch's setup chain is emitted mid-loop so its DMAs and
        # transposes hide under the current batch's compute.
        PT0 = emit_setup_a(0)
        XU0 = emit_setup_b(PT0)
        tiles = (XU0,) + emit_setup_c(XU0)
        A2_cur = build_a2(tiles, 0)
        pipelined = None  # (b, ACC, H1, it) -- lags one iteration, across batches
        for b in range(N_BATCH):
            ACC = opool.tile([128, 9], f32, tag="acc")
            next_pt = next_xu = next_tiles = None
            for it in range(nit):
                A2 = A2_cur
                if it + 1 < nit:
                    A2_cur = build_a2(tiles, it + 1)
                H1 = emit_p1(tiles, A2, it)
                if pipelined is not None:
                    pb, pacc, ph1, pit = pipelined
                    emit_layer2(pacc, ph1, pit)
                    if pit == nit - 1:
                        nc.sync.dma_start(acc_o[pb], pacc[:])
                pipelined = (b, ACC, H1, it)
                if b + 1 < N_BATCH:
                    if it == nit - 8:
                        next_pt = emit_setup_a(b + 1)
                    elif it == nit - 6:
                        next_xu = emit_setup_b(next_pt)
                    elif it == nit - 4:
                        next_tiles = (next_xu,) + emit_setup_c(next_xu)
                    elif it == nit - 1:
                        A2_cur = build_a2(next_tiles, 0)
            if next_tiles is not None:
                tiles = next_tiles
        pb, pacc, ph1, pit = pipelined
        emit_layer2(pacc, ph1, pit)
        nc.sync.dma_start(acc_o[pb], pacc[:])

    nc.compile()
    return nc


def _get_program():
    key = (N_DUALS, N_BATCH)
    if key not in _CACHE:
        _CACHE[key] = _build_program()
    return _CACHE[key]


def _get_runner():
    """Build (once) a cached jitted SPMD executable for the program."""
    key = ("runner", N_DUALS, N_BATCH, N_RUN_CORES)
    if key in _CACHE:
        return _CACHE[key]
    import jax
    import jax.numpy as jnp
    import numpy as _np
    import concourse.mybir as mybir
    from jax.sharding import Mesh, PartitionSpec
    from jax.experimental.shard_map import shard_map
    from concourse import bass2jax
    from concourse.bass2jax import _bass_exec_p, partition_id_tensor

    bass2jax.install_neuronx_cc_hook()
    nc = _get_program()
    n_cores = N_RUN_CORES

    partition_name = (nc.partition_id_tensor.name
                      if nc.partition_id_tensor else None)
    in_names, out_names, out_avals, zero_shapes = [], [], [], []
    for alloc in nc.m.functions[0].allocations:
        if not isinstance(alloc, mybir.MemoryLocationSet):
            continue
        name = alloc.memorylocations[0].name
        if alloc.kind == "ExternalInput":
            if name != partition_name:
                in_names.append(name)
        elif alloc.kind == "ExternalOutput":
            out_names.append(name)
            shape = tuple(alloc.tensor_shape)
            dtype = mybir.dt.np(alloc.dtype)
            out_avals.append(jax.core.ShapedArray(shape, dtype))
            zero_shapes.append((shape, dtype))
    n_params = len(in_names)
    n_outs = len(out_avals)
    all_in_names = list(in_names) + list(out_names)
    if partition_name is not None:
        all_in_names.append(partition_name)
    donate = tuple(range(n_params, n_params + n_outs))

    def _body(*args):
        operands = list(args)
        if partition_name is not None:
            operands.append(partition_id_tensor())
        outs = _bass_exec_p.bind(
            *operands, out_avals=tuple(out_avals), in_names=tuple(all_in_names),
            out_names=tuple(out_names), lowering_input_output_aliases=(),
            sim_require_finite=True, sim_require_nnan=True, nc=nc)
        return tuple(outs)

    devices = jax.devices()[:n_cores]
    mesh = Mesh(_np.asarray(devices), ("core",))
    in_specs = (PartitionSpec("core"),) * (n_params + n_outs)
    out_specs = (PartitionSpec("core"),) * len(out_names)
    sharded = jax.jit(
        shard_map(_body, mesh=mesh, in_specs=in_specs, out_specs=out_specs,
                  check_rep=False),
        donate_argnums=donate, keep_unused=True)

    def run(in_maps):
        concat_in = [
            np.concatenate([np.asarray(in_maps[c][nm]) for c in range(n_cores)],
                           axis=0)
            for nm in in_names
        ]
        concat_zeros = [np.zeros((n_cores * s[0], *s[1:]), d)
                        for (s, d) in zero_shapes]
        out_arrs = sharded(*concat_in, *concat_zeros)
        return [
            {nm: np.asarray(out_arrs[i]).reshape(n_cores, *out_avals[i].shape)[c]
             for i, nm in enumerate(out_names)}
            for c in range(n_cores)
        ]

    _CACHE[key] = run
    return run


def _make_in_maps(emb, tw, w1, b1, w2, b2):
    import ml_dtypes

    emb = np.asarray(emb, np.float32)
    tw = np.asarray(tw, np.float32)
    w1 = np.asarray(w1, np.float32)
    b1v = np.asarray(b1, np.float32)
    b2v = np.asarray(b2, np.float32)

    bfl = ml_dtypes.bfloat16
    w1bt = np.ascontiguousarray(w1[:, 64:].T).astype(bfl)          # [64, 64]
    w1at = np.ascontiguousarray(0.5 * w1[:, :64].T).astype(bfl)    # [64, 64]
    w2f = np.asarray(w2, np.float32)
    # W2P8 [128, 128]: strip cb's [128, 32] weight: contraction rows 0:64
    # (H1 top offset) output at local cols 0:16, rows 64:128 at 16:32.
    w2p8 = np.zeros((128, 128), np.float32)
    for cb in range(4):
        w2p8[0:64, 32 * cb:32 * cb + 16] = w2f.T
        w2p8[64:128, 32 * cb + 16:32 * cb + 32] = w2f.T
    w2p8 = w2p8.astype(bfl)
    b1s = np.concatenate([b1v, b1v]).reshape(128, 1).astype(np.float32)
    b2s8 = np.tile(b2v, 8).reshape(128, 1).astype(np.float32)
    twp = np.ascontiguousarray(tw[:-1].reshape(L, D)).astype(np.float32)
    twl = np.array([[tw[-1]]], np.float32)

    shared = {
        "w1bt": w1bt, "w1at": w1at, "w2p8": w2p8,
        "b1s": b1s, "b2s8": b2s8, "twp": twp, "twl": twl,
    }
    in_maps = []
    for c in range(N_CORES):
        m = dict(shared)
        m["emb4"] = np.ascontiguousarray(emb[c * B_PER_CORE:(c + 1) * B_PER_CORE])
        in_maps.append(m)
    return in_maps


def kernel(emb, tw, tb, w1, b1, w2, b2, w3, b3, scale):
    run = _get_runner()
    in_maps = _make_in_maps(emb, tw, w1, b1, w2, b2)
    core_results = run(in_maps[:N_RUN_CORES])

    w3v = np.asarray(w3, np.float32)[0]
    out = np.zeros(32, np.float32)
    for c in range(N_RUN_CORES):
        r = core_results[c]
        acc = r["acc_o"]            # [4, 128, 16]
        x1p = r["x1_o"][:, 0, 0]    # [4]
        for b in range(N_BATCH):
            m16 = acc[b].reshape(128, 9)
            R = m16.reshape(8, 16, 9).sum(axis=(0, 2))
            # o=128 (col 8, partitions 96:112) was double counted
            R -= 0.5 * m16[96:112, 8]
            out[c * B_PER_CORE + b] = (
                x1p[b] + float(tb[0])
                + float(scale[0]) * (R @ w3v + float(b3[0]) * NPAIRS)
            )
    return out



# revision 27
# speedup vs baseline: 1.0083x; 1.0083x over previous
"""Trainium2 Bass kernel for nn_EpiNN_aaindex (pairwise-MLP GNN reduction).

Math (per batch b):
  x1 = emb@tw + tb
  X[i,d] = emb[i*64+d] * tw[i*64+d]            (L=256, D=64)
  s_ij = MLP(concat[(x_i+x_j)/2, |x_i-x_j|])   (64->16->1, LeakyReLU 0.01)
  out_b = x1 + scale * sum_{i<j} s_ij

Strategy: 8 cores, 4 batches/core (data parallel over B=32).
Exact upper-triangle enumeration via cyclic offsets o=1..128:
pairs (i, (i+o) mod 256) for o=1..127 cover each unordered pair once;
o=128 covers each of its 128 pairs twice (weighted 0.5 on the host).

Layouts (per batch, SBUF, bf16):
  XU  [128, 512]: both partition halves = [X2T | X2T]  (X2T = X.T [64, 256])
  XSo [128, 512]: top = X2T shifted 1, bottom = shifted 65   (odd offsets)
  XSe [128, 512]: top = X2T shifted 2, bottom = shifted 66   (even offsets)
Unit u = offset pair (u, u+64); dual-unit d = units (2d+1, 2d+2).
A2 [128, 512] = |x_i - x_j| features for 4 offsets (2 per partition half).
P1 [128, 512] psum quadrants = pre1 for the 4 offsets (3 matmuls each:
  w1b@A, 0.5*w1a@X2T (u_i term), 0.5*w1a@X2T-shifted (u_j term)).
ACT Lrelu (+b1 bias) -> H1 bf16 -> 4 layer-2 matmuls -> P2 -> ACT Lrelu
(+b2) with accum_out giving the free-dim (i) sums for free.

Final combine on host: out = x1 + scale*(w3 . R + 32640*b3).
"""
import numpy as np

L, D = 256, 64
B_PER_CORE = 4
N_CORES = 8
NPAIRS = 32640  # 256*255/2

_CACHE = {}
import os as _os
N_DUALS = int(_os.environ.get("EPINN_DUALS", "32"))
N_BATCH = int(_os.environ.get("EPINN_BATCH", str(B_PER_CORE)))
N_RUN_CORES = int(_os.environ.get("EPINN_CORES", str(N_CORES)))
STAGE = int(_os.environ.get("EPINN_STAGE", "9"))
VAR = _os.environ.get("EPINN_VAR", "")


def _build_program():
    import concourse.bacc as bacc
    import concourse.bass as bass
    import concourse.mybir as mybir
    import concourse.tile as tile
    from contextlib import ExitStack

    f32 = mybir.dt.float32
    bf16 = mybir.dt.bfloat16
    u16 = mybir.dt.uint16
    AF = mybir.ActivationFunctionType
    ALU = mybir.AluOpType
    AF_MAIN = AF.Relu if _os.environ.get("EPINN_RELU") else AF.Lrelu

    nc = bacc.Bacc("TRN2", target_bir_lowering=False, debug=False,
                   num_devices=N_CORES)

    # ---- DRAM parameters (per core) ----
    emb_d = nc.declare_dram_parameter("emb4", [B_PER_CORE, L * D + 1], f32,
                                      isOutput=False)
    w1bt_d = nc.declare_dram_parameter("w1bt", [64, 64], bf16, isOutput=False)
    w1at_d = nc.declare_dram_parameter("w1at", [64, 64], bf16, isOutput=False)
    w2p8_d = nc.declare_dram_parameter("w2p8", [128, 128], bf16, isOutput=False)
    b1s_d = nc.declare_dram_parameter("b1s", [128, 1], f32, isOutput=False)
    b2s8_d = nc.declare_dram_parameter("b2s8", [128, 1], f32, isOutput=False)
    twp_d = nc.declare_dram_parameter("twp", [L, D], f32, isOutput=False)
    twl_d = nc.declare_dram_parameter("twl", [1, 1], f32, isOutput=False)

    acc_o = nc.declare_dram_parameter("acc_o", [B_PER_CORE, 128, 9], f32,
                                      isOutput=True)
    x1_o = nc.declare_dram_parameter("x1_o", [B_PER_CORE, 1, 1], f32,
                                     isOutput=True)

    with tile.TileContext(nc) as tc, ExitStack() as ctx:
        cpool = ctx.enter_context(tc.tile_pool(name="consts", bufs=1))
        xpool = ctx.enter_context(tc.tile_pool(name="xbufs", bufs=2))
        apool = ctx.enter_context(tc.tile_pool(name="abufs", bufs=3))
        hpool = ctx.enter_context(tc.tile_pool(name="hbufs", bufs=3))
        jpool = ctx.enter_context(tc.tile_pool(name="junk", bufs=2))
        opool = ctx.enter_context(tc.tile_pool(name="outs", bufs=2))
        pp1 = ctx.enter_context(tc.tile_pool(name="p1", bufs=2, space="PSUM"))
        pp2 = ctx.enter_context(tc.tile_pool(name="p2", bufs=2, space="PSUM"))
        ppt = ctx.enter_context(tc.tile_pool(name="pt", bufs=1, space="PSUM"))
        ppx = ctx.enter_context(tc.tile_pool(name="px", bufs=1, space="PSUM"))

        # ---- static weights / consts ----
        WB = cpool.tile([128, 64], bf16)
        WA = cpool.tile([128, 64], bf16)
        W2P8 = cpool.tile([128, 128], bf16)
        B1S = cpool.tile([128, 1], f32)
        B2S8 = cpool.tile([128, 1], f32)
        TWP = cpool.tile([128, 2, 64], f32)  # [128p, (half, d)]
        TWL = cpool.tile([1, 1], f32)
        IDENT = cpool.tile([128, 128], f32)
        ONES = cpool.tile([128, 1], f32)

        E2_0 = xpool.tile([128, 2, 64], f32, tag="e2")
        nc.sync.dma_start(
            E2_0[:], emb_d[0, 0:L * D].rearrange("(h p f) -> p h f", p=128, f=64)
        )
        nc.gpsimd.dma_start(TWP[:, 0, :], twp_d[0:128, :])
        nc.gpsimd.dma_start(TWP[:, 1, :], twp_d[128:256, :])
        nc.sync.dma_start(WB[0:64, :], w1bt_d[:])
        nc.sync.dma_start(WB[64:128, :], w1bt_d[:])
        nc.sync.dma_start(WA[0:64, :], w1at_d[:])
        nc.sync.dma_start(WA[64:128, :], w1at_d[:])
        nc.sync.dma_start(W2P8[:], w2p8_d[:])
        nc.scalar.dma_start(B1S[:], b1s_d[:])
        nc.scalar.dma_start(B2S8[:], b2s8_d[:])
        nc.scalar.dma_start(TWL[:], twl_d[:])
        nc.gpsimd.memset(IDENT[:], 0.0)
        nc.gpsimd.affine_select(
            out=IDENT[:], in_=IDENT[:], compare_op=ALU.not_equal, fill=1.0,
            base=0, pattern=[[-1, 128]], channel_multiplier=1,
        )
        nc.gpsimd.memset(ONES[:], 1.0)

        nit = N_DUALS // 2

        def emit_setup_a(b):
            """emb load, X = emb*tw, x1 scalar path, transpose to PT."""
            if b == 0:
                E2 = E2_0
            else:
                E2 = xpool.tile([128, 2, 64], f32, tag="e2")
                nc.sync.dma_start(
                    E2[:], emb_d[b, 0:L * D].rearrange("(h p f) -> p h f",
                                                       p=128, f=64)
                )
            EL = xpool.tile([1, 1], f32, tag="el")
            nc.sync.dma_start(EL[:], emb_d[b, L * D:L * D + 1][None, :])

            X2F = xpool.tile([128, 2, 64], f32, tag="x2f")
            nc.vector.tensor_tensor(out=X2F[:], in0=E2[:], in1=TWP[:],
                                    op=ALU.mult)

            # x1 = sum(X2F) + EL*twl + tb  (tb added on host)
            CS = xpool.tile([128, 2], f32, tag="cs")
            nc.vector.tensor_reduce(out=CS[:, 0:1], in_=X2F[:, 0, :],
                                    op=ALU.add, axis=mybir.AxisListType.X)
            nc.vector.tensor_reduce(out=CS[:, 1:2], in_=X2F[:, 1, :],
                                    op=ALU.add, axis=mybir.AxisListType.X)
            CS1 = xpool.tile([128, 1], f32, tag="cs1")
            nc.vector.tensor_tensor(out=CS1[:], in0=CS[:, 0:1], in1=CS[:, 1:2],
                                    op=ALU.add)
            PX1 = ppx.tile([1, 1], f32, tag="px1")
            nc.tensor.matmul(PX1[:], CS1[:], ONES[:], start=True, stop=False,
                             skip_group_check=True)
            nc.tensor.matmul(PX1[:], EL[:], TWL[:], start=False, stop=True,
                             skip_group_check=True)
            X1S = xpool.tile([1, 1], f32, tag="x1s")
            nc.vector.tensor_scalar(out=X1S[:], in0=PX1[:], scalar1=1.0,
                                    scalar2=None, op0=ALU.mult)
            nc.sync.dma_start(x1_o[b], X1S[:])

            # transpose X -> X2T [64d, 256i] in psum
            PT = ppt.tile([64, 256], f32, tag="pt")
            nc.tensor.matmul(PT[:, 0:128], X2F[:, 0, :], IDENT[:],
                             is_transpose=True, start=True, stop=True,
                             skip_group_check=True)
            nc.tensor.matmul(PT[:, 128:256], X2F[:, 1, :], IDENT[:],
                             is_transpose=True, start=True, stop=True,
                             skip_group_check=True)
            return PT

        def emit_setup_b(PT):
            """XU tile: [X2T | X2T] on both partition halves."""
            XU = xpool.tile([128, 512], bf16, tag="xu")
            nc.vector.tensor_scalar(out=XU[0:64, 0:256], in0=PT[:],
                                    scalar1=1.0, scalar2=None, op0=ALU.mult)
            nc.sync.dma_start(XU[0:64, 256:512], XU[0:64, 0:256])
            nc.sync.dma_start(XU[64:128, :], XU[0:64, :])
            return XU

        def emit_setup_c(XU):
            """XSo/XSe shifted tiles (spread across scalar/gpsimd queues)."""
            XSo = xpool.tile([128, 512], bf16, tag="xso")
            XSe = xpool.tile([128, 512], bf16, tag="xse")
            # XSo: top shift 1, bottom shift 65
            nc.scalar.dma_start(XSo[0:64, 0:511], XU[0:64, 1:512])
            nc.scalar.dma_start(XSo[0:64, 511:512], XU[0:64, 255:256])
            nc.scalar.dma_start(XSo[64:128, 0:447], XU[0:64, 65:512])
            nc.scalar.dma_start(XSo[64:128, 447:512], XU[0:64, 0:65])
            # XSe: top shift 2, bottom shift 66
            nc.gpsimd.dma_start(XSe[0:64, 0:510], XU[0:64, 2:512])
            nc.gpsimd.dma_start(XSe[0:64, 510:512], XU[0:64, 254:256])
            nc.gpsimd.dma_start(XSe[64:128, 0:446], XU[0:64, 66:512])
            nc.gpsimd.dma_start(XSe[64:128, 446:512], XU[0:64, 0:66])
            return XSo, XSe

        def build_a2(tiles, it):
            XU, XSo, XSe = tiles
            A2 = apool.tile([128, 1024], bf16, tag="a2")
            # cols 0:256   = shift (4it+1 | +65)   from XSo offset 4it
            # cols 256:512 = shift (4it+3 | +67)   from XSo offset 4it+2
            # cols 512:768 = shift (4it+2 | +66)   from XSe offset 4it
            # cols 768:1024= shift (4it+4 | +68)   from XSe offset 4it+2
            c0 = 4 * it
            in0b = XU[:, 0:256].unsqueeze(1).broadcast_to([128, 2, 256])
            for (dst, src) in ((A2[:, 0:512], XSo), (A2[:, 512:1024], XSe)):
                sl = src[:, c0:c0 + 258]
                in1w = bass.AP(tensor=sl.tensor, offset=sl.offset,
                               ap=[list(sl.ap[0]), [2, 2], [1, 256]])
                nc.vector.tensor_tensor(
                    out=dst.rearrange("p (a b) -> p a b", a=2),
                    in0=in0b, in1=in1w, op=ALU.subtract)
            nc.vector.tensor_scalar(
                out=A2[:].bitcast(u16), in0=A2[:].bitcast(u16),
                scalar1=0x7FFF, scalar2=None, op0=ALU.bitwise_and)
            return A2

        def emit_p1(tiles, A2, it):
            XU, XSo, XSe = tiles
            c0 = 4 * it
            # P1a: T0 (rows 0:64 <- o=u1,u1p) T10 (rows 64:128 <- +64)
            # P1b: T8 (rows 0:64 <- o=u2+64,u2p+64) T2 (rows 64:128 <- u2,u2p)
            P1 = pp1.tile([128, 1024], f32, tag="p1")
            for (pc, tp, ar) in (
                (0, (0, 0), 0),        # T0
                (0, (64, 64), 64),     # T10
                (512, (64, 0), 64),    # T8  (array rows 64-127 -> psum 0-63)
                (512, (0, 64), 0),     # T2  (array rows 0-63 -> psum 64-127)
            ):
                rg, pr = ar, tp[1]
                XSx = XSo if pc == 0 else XSe
                ps = P1[pr:pr + 64, pc:pc + 512]
                nc.tensor.matmul(ps, WB[rg:rg + 64, :], A2[rg:rg + 64, pc:pc + 512],
                                 start=True, stop=False, tile_position=tp,
                                 skip_group_check=True)
                nc.tensor.matmul(ps, WA[rg:rg + 64, :],
                                 XU[rg:rg + 64, 0:512],
                                 start=False, stop=False, tile_position=tp,
                                 skip_group_check=True)
                nc.tensor.matmul(ps[:, 0:256], WA[rg:rg + 64, :],
                                 XSx[rg:rg + 64, c0:c0 + 256],
                                 start=False, stop=False, tile_position=tp,
                                 skip_group_check=True)
                nc.tensor.matmul(ps[:, 256:512], WA[rg:rg + 64, :],
                                 XSx[rg:rg + 64, c0 + 2:c0 + 258],
                                 start=False, stop=True, tile_position=tp,
                                 skip_group_check=True)
            H1 = hpool.tile([128, 1024], bf16, tag="h1")
            nc.scalar.activation(H1[:], P1[:], AF_MAIN, bias=B1S[:],
                                 scale=1.0, alpha=0.01)
            return H1

        p2state = [None]

        def emit_layer2(ACC, H1, it):
            # layer 2: pack 8 offsets x 16 outputs across 128 psum
            # partitions; P2 free dim is only 256 -> 4x cheaper lrelu2.
            # One 128-contraction MM per 32-partition strip: weight rows
            # 0:64 (H1 top offset) land in cols 0:16, rows 64:128 in
            # 16:32 -- avoids two concurrent drains on one strip.
            # Two consecutive iterations share one 2-bank P2F tile (cols
            # 0:256 / 256:512) so lrelu2 + reduce run once per pair.
            # it=14 and the o=128 iteration (15) stay unpaired so the host
            # can halve the double-counted o=128 block (ACC cols 7 and 8).
            if it == 15:
                pair, po, col = (15,), 0, 8
            elif it == 14:
                pair, po, col = (14,), 0, 7
            else:
                pair, po, col = (it - it % 2, it - it % 2 + 1), 256 * (it % 2), it // 2
            if po == 0:
                P2F = pp2.tile([128, 512], f32, tag="p2")
                p2state[0] = P2F
            else:
                P2F = p2state[0]
            for cb in range(4):
                hc = 256 * cb
                nc.tensor.matmul(P2F[32 * cb:32 * cb + 32, po:po + 256],
                                 W2P8[:, 32 * cb:32 * cb + 32],
                                 H1[:, hc:hc + 256],
                                 start=True, stop=True,
                                 tile_position=(0, 32 * cb),
                                 skip_group_check=True)
            if it != pair[-1]:
                return
            n = 256 * len(pair)
            # lrelu2 on ACT (bias free), column-sum over i on DVE
            H2 = jpool.tile([128, 512], bf16, tag="h2")
            nc.scalar.activation(H2[:, 0:n], P2F[:, 0:n], AF_MAIN,
                                 bias=B2S8[:], scale=1.0, alpha=0.01)
            nc.vector.tensor_reduce(out=ACC[:, col:col + 1], in_=H2[:, 0:n],
                                    op=ALU.add, axis=mybir.AxisListType.X)

        # ---- main loops: 16 iterations x 8 offsets per batch ----
        # offsets at iteration it: T0 stream: u1=4it+1, u1p=4it+3
        #                          T2/T8 stream: u2=4it+2, u2p=4it+4
        #                          +64 variants on the bottom halves
        # Software-pipelined so the PE queue never stalls: A2 is built one
        # iteration ahead (DVE overlaps P1[it]); layer2[it-1] (whose MMs wait
        # on act1[it-1], finished during P1[it]) is emitted after act1[it];
        # the NEXT batch's setup chain is emitted mid-loop so its DMAs and
        # transposes hide under the current batch's compute.
        PT0 = emit_setup_a(0)
        XU0 = emit_setup_b(PT0)
        tiles = (XU0,) + emit_setup_c(XU0)
        A2_cur = build_a2(tiles, 0)
        pipelined = None  # (b, ACC, H1, it) -- lags one iteration, across batches
        for b in range(N_BATCH):
            ACC = opool.tile([128, 9], f32, tag="acc")
            next_pt = next_xu = next_tiles = None
            for it in range(nit):
                A2 = A2_cur
                if it + 1 < nit:
                    A2_cur = build_a2(tiles, it + 1)
                H1 = emit_p1(tiles, A2, it)
                if pipelined is not None:
                    pb, pacc, ph1, pit = pipelined
                    emit_layer2(pacc, ph1, pit)
                    if pit == nit - 1:
                        nc.sync.dma_start(acc_o[pb], pacc[:])
                pipelined = (b, ACC, H1, it)
                if b + 1 < N_BATCH:
                    if it == 1:
                        next_pt = emit_setup_a(b + 1)
                    elif it == 3:
                        next_xu = emit_setup_b(next_pt)
                    elif it == 5:
                        next_tiles = (next_xu,) + emit_setup_c(next_xu)
                    elif it == nit - 1:
                        A2_cur = build_a2(next_tiles, 0)
            if next_tiles is not None:
                tiles = next_tiles
        pb, pacc, ph1, pit = pipelined
        emit_layer2(pacc, ph1, pit)
        nc.sync.dma_start(acc_o[pb], pacc[:])

    nc.compile()
    return nc


def _get_program():
    key = (N_DUALS, N_BATCH)
    if key not in _CACHE:
        _CACHE[key] = _build_program()
    return _CACHE[key]


def _get_runner():
    """Build (once) a cached jitted SPMD executable for the program."""
    key = ("runner", N_DUALS, N_BATCH, N_RUN_CORES)
    if key in _CACHE:
        return _CACHE[key]
    import jax
    import jax.numpy as jnp
    import numpy as _np
    import concourse.mybir as mybir
    from jax.sharding import Mesh, PartitionSpec
    from jax.experimental.shard_map import shard_map
    from concourse import bass2jax
    from concourse.bass2jax import _bass_exec_p, partition_id_tensor

    bass2jax.install_neuronx_cc_hook()
    nc = _get_program()
    n_cores = N_RUN_CORES

    partition_name = (nc.partition_id_tensor.name
                      if nc.partition_id_tensor else None)
    in_names, out_names, out_avals, zero_shapes = [], [], [], []
    for alloc in nc.m.functions[0].allocations:
        if not isinstance(alloc, mybir.MemoryLocationSet):
            continue
        name = alloc.memorylocations[0].name
        if alloc.kind == "ExternalInput":
            if name != partition_name:
                in_names.append(name)
        elif alloc.kind == "ExternalOutput":
            out_names.append(name)
            shape = tuple(alloc.tensor_shape)
            dtype = mybir.dt.np(alloc.dtype)
            out_avals.append(jax.core.ShapedArray(shape, dtype))
            zero_shapes.append((shape, dtype))
    n_params = len(in_names)
    n_outs = len(out_avals)
    all_in_names = list(in_names) + list(out_names)
    if partition_name is not None:
        all_in_names.append(partition_name)
    donate = tuple(range(n_params, n_params + n_outs))

    def _body(*args):
        operands = list(args)
        if partition_name is not None:
            operands.append(partition_id_tensor())
        outs = _bass_exec_p.bind(
            *operands, out_avals=tuple(out_avals), in_names=tuple(all_in_names),
            out_names=tuple(out_names), lowering_input_output_aliases=(),
            sim_require_finite=True, sim_require_nnan=True, nc=nc)
        return tuple(outs)

    devices = jax.devices()[:n_cores]
    mesh = Mesh(_np.asarray(devices), ("core",))
    in_specs = (PartitionSpec("core"),) * (n_params + n_outs)
    out_specs = (PartitionSpec("core"),) * len(out_names)
    sharded = jax.jit(
        shard_map(_body, mesh=mesh, in_specs=in_specs, out_specs=out_specs,
                  check_rep=False),
        donate_argnums=donate, keep_unused=True)

    def run(in_maps):
        concat_in = [
            np.concatenate([np.asarray(in_maps[c][nm]) for c in range(n_cores)],
                           axis=0)
            for nm in in_names
        ]
        concat_zeros = [np.zeros((n_cores * s[0], *s[1:]), d)
                        for (s, d) in zero_shapes]
        out_arrs = sharded(*concat_in, *concat_zeros)
        return [
            {nm: np.asarray(out_arrs[i]).reshape(n_cores, *out_avals[i].shape)[c]
             for i, nm in enumerate(out_names)}
            for c in range(n_cores)
        ]

    _CACHE[key] = run
    return run


def _make_in_maps(emb, tw, w1, b1, w2, b2):
    import ml_dtypes

    emb = np.asarray(emb, np.float32)
    tw = np.asarray(tw, np.float32)
    w1 = np.asarray(w1, np.float32)
    b1v = np.asarray(b1, np.float32)
    b2v = np.asarray(b2, np.float32)

    bfl = ml_dtypes.bfloat16
    w1bt = np.ascontiguousarray(w1[:, 64:].T).astype(bfl)          # [64, 64]
    w1at = np.ascontiguousarray(0.5 * w1[:, :64].T).astype(bfl)    # [64, 64]
    w2f = np.asarray(w2, np.float32)
    # W2P8 [128, 128]: strip cb's [128, 32] weight: contraction rows 0:64
    # (H1 top offset) output at local cols 0:16, rows 64:128 at 16:32.
    w2p8 = np.zeros((128, 128), np.float32)
    for cb in range(4):
        w2p8[0:64, 32 * cb:32 * cb + 16] = w2f.T
        w2p8[64:128, 32 * cb + 16:32 * cb + 32] = w2f.T
    w2p8 = w2p8.astype(bfl)
    b1s = np.concatenate([b1v, b1v]).reshape(128, 1).astype(np.float32)
    b2s8 = np.tile(b2v, 8).reshape(128, 1).astype(np.float32)
    twp = np.ascontiguousarray(tw[:-1].reshape(L, D)).astype(np.float32)
    twl = np.array([[tw[-1]]], np.float32)

    shared = {
        "w1bt": w1bt, "w1at": w1at, "w2p8": w2p8,
        "b1s": b1s, "b2s8": b2s8, "twp": twp, "twl": twl,
    }
    in_maps = []
    for c in range(N_CORES):
        m = dict(shared)
        m["emb4"] = np.ascontiguousarray(emb[c * B_PER_CORE:(c + 1) * B_PER_CORE])
        in_maps.append(m)
    return in_maps


def kernel(emb, tw, tb, w1, b1, w2, b2, w3, b3, scale):
    run = _get_runner()
    in_maps = _make_in_maps(emb, tw, w1, b1, w2, b2)
    core_results = run(in_maps[:N_RUN_CORES])

    w3v = np.asarray(w3, np.float32)[0]
    out = np.zeros(32, np.float32)
    for c in range(N_RUN_CORES):
        r = core_results[c]
        acc = r["acc_o"]            # [4, 128, 16]
        x1p = r["x1_o"][:, 0, 0]    # [4]
        for b in range(N_BATCH):
            m16 = acc[b].reshape(128, 9)
            R = m16.reshape(8, 16, 9).sum(axis=(0, 2))
            # o=128 (col 8, partitions 96:112) was double counted
            R -= 0.5 * m16[96:112, 8]
            out[c * B_PER_CORE + b] = (
                x1p[b] + float(tb[0])
                + float(scale[0]) * (R @ w3v + float(b3[0]) * NPAIRS)
            )
    return out



# revision 28
# speedup vs baseline: 1.0608x; 1.0521x over previous
"""Trainium2 Bass kernel for nn_EpiNN_aaindex (pairwise-MLP GNN reduction).

Math (per batch b):
  x1 = emb@tw + tb
  X[i,d] = emb[i*64+d] * tw[i*64+d]            (L=256, D=64)
  s_ij = MLP(concat[(x_i+x_j)/2, |x_i-x_j|])   (64->16->1, LeakyReLU 0.01)
  out_b = x1 + scale * sum_{i<j} s_ij

Strategy: 8 cores, 4 batches/core (data parallel over B=32).
Exact upper-triangle enumeration via cyclic offsets o=1..128:
pairs (i, (i+o) mod 256) for o=1..127 cover each unordered pair once;
o=128 covers each of its 128 pairs twice (weighted 0.5 on the host).

Layouts (per batch, SBUF, bf16):
  XU  [128, 512]: both partition halves = [X2T | X2T]  (X2T = X.T [64, 256])
  XSo [128, 512]: top = X2T shifted 1, bottom = shifted 65   (odd offsets)
  XSe [128, 512]: top = X2T shifted 2, bottom = shifted 66   (even offsets)
Unit u = offset pair (u, u+64); dual-unit d = units (2d+1, 2d+2).
A2 [128, 512] = |x_i - x_j| features for 4 offsets (2 per partition half).
P1 [128, 512] psum quadrants = pre1 for the 4 offsets (3 matmuls each:
  w1b@A, 0.5*w1a@X2T (u_i term), 0.5*w1a@X2T-shifted (u_j term)).
ACT Lrelu (+b1 bias) -> H1 bf16 -> 4 layer-2 matmuls -> P2 -> ACT Lrelu
(+b2) with accum_out giving the free-dim (i) sums for free.

Final combine on host: out = x1 + scale*(w3 . R + 32640*b3).
"""
import numpy as np

L, D = 256, 64
B_PER_CORE = 4
N_CORES = 8
NPAIRS = 32640  # 256*255/2

_CACHE = {}
import os as _os
N_DUALS = int(_os.environ.get("EPINN_DUALS", "32"))
N_BATCH = int(_os.environ.get("EPINN_BATCH", str(B_PER_CORE)))
N_RUN_CORES = int(_os.environ.get("EPINN_CORES", str(N_CORES)))
STAGE = int(_os.environ.get("EPINN_STAGE", "9"))
VAR = _os.environ.get("EPINN_VAR", "")


def _build_program():
    import concourse.bacc as bacc
    import concourse.bass as bass
    import concourse.mybir as mybir
    import concourse.tile as tile
    from contextlib import ExitStack

    f32 = mybir.dt.float32
    bf16 = mybir.dt.bfloat16
    u16 = mybir.dt.uint16
    AF = mybir.ActivationFunctionType
    ALU = mybir.AluOpType
    AF_MAIN = AF.Relu if _os.environ.get("EPINN_RELU") else AF.Lrelu

    nc = bacc.Bacc("TRN2", target_bir_lowering=False, debug=False,
                   num_devices=N_CORES)

    # ---- DRAM parameters (per core) ----
    emb_d = nc.declare_dram_parameter("emb4", [B_PER_CORE, L * D + 1], f32,
                                      isOutput=False)
    w1bt_d = nc.declare_dram_parameter("w1bt", [64, 64], bf16, isOutput=False)
    w1at_d = nc.declare_dram_parameter("w1at", [64, 64], bf16, isOutput=False)
    w2p8_d = nc.declare_dram_parameter("w2p8", [128, 128], bf16, isOutput=False)
    b1s_d = nc.declare_dram_parameter("b1s", [128, 1], f32, isOutput=False)
    b2s8_d = nc.declare_dram_parameter("b2s8", [128, 1], f32, isOutput=False)
    twp_d = nc.declare_dram_parameter("twp", [L, D], f32, isOutput=False)
    twl_d = nc.declare_dram_parameter("twl", [1, 1], f32, isOutput=False)

    acc_o = nc.declare_dram_parameter("acc_o", [B_PER_CORE, 128, 9], f32,
                                      isOutput=True)
    x1_o = nc.declare_dram_parameter("x1_o", [B_PER_CORE, 1, 1], f32,
                                     isOutput=True)

    with tile.TileContext(nc) as tc, ExitStack() as ctx:
        cpool = ctx.enter_context(tc.tile_pool(name="consts", bufs=1))
        xpool = ctx.enter_context(tc.tile_pool(name="xbufs", bufs=2))
        apool = ctx.enter_context(tc.tile_pool(name="abufs", bufs=3))
        hpool = ctx.enter_context(tc.tile_pool(name="hbufs", bufs=3))
        jpool = ctx.enter_context(tc.tile_pool(name="junk", bufs=2))
        opool = ctx.enter_context(tc.tile_pool(name="outs", bufs=2))
        pp1 = ctx.enter_context(tc.tile_pool(name="p1", bufs=2, space="PSUM"))
        pp2 = ctx.enter_context(tc.tile_pool(name="p2", bufs=2, space="PSUM"))
        ppt = ctx.enter_context(tc.tile_pool(name="pt", bufs=1, space="PSUM"))
        ppx = ctx.enter_context(tc.tile_pool(name="px", bufs=1, space="PSUM"))

        # ---- static weights / consts ----
        WB = cpool.tile([128, 64], bf16)
        WA = cpool.tile([128, 64], bf16)
        W2P8 = cpool.tile([128, 128], bf16)
        B1S = cpool.tile([128, 1], f32)
        B2S8 = cpool.tile([128, 1], f32)
        TWP = cpool.tile([128, 2, 64], f32)  # [128p, (half, d)]
        TWL = cpool.tile([1, 1], f32)
        IDENT = cpool.tile([128, 128], f32)
        ONES = cpool.tile([128, 1], f32)

        E2_0 = xpool.tile([128, 2, 64], f32, tag="e2")
        nc.sync.dma_start(
            E2_0[:], emb_d[0, 0:L * D].rearrange("(h p f) -> p h f", p=128, f=64)
        )
        nc.gpsimd.dma_start(TWP[:, 0, :], twp_d[0:128, :])
        nc.gpsimd.dma_start(TWP[:, 1, :], twp_d[128:256, :])
        nc.sync.dma_start(WB[0:64, :], w1bt_d[:])
        nc.sync.dma_start(WB[64:128, :], w1bt_d[:])
        nc.sync.dma_start(WA[0:64, :], w1at_d[:])
        nc.sync.dma_start(WA[64:128, :], w1at_d[:])
        nc.sync.dma_start(W2P8[:], w2p8_d[:])
        nc.scalar.dma_start(B1S[:], b1s_d[:])
        nc.scalar.dma_start(B2S8[:], b2s8_d[:])
        nc.scalar.dma_start(TWL[:], twl_d[:])
        nc.gpsimd.memset(IDENT[:], 0.0)
        nc.gpsimd.affine_select(
            out=IDENT[:], in_=IDENT[:], compare_op=ALU.not_equal, fill=1.0,
            base=0, pattern=[[-1, 128]], channel_multiplier=1,
        )
        nc.gpsimd.memset(ONES[:], 1.0)

        nit = N_DUALS // 2

        def emit_setup_a(b):
            """emb load, X = emb*tw, x1 scalar path, transpose to PT."""
            if b == 0:
                E2 = E2_0
            else:
                E2 = xpool.tile([128, 2, 64], f32, tag="e2")
                nc.sync.dma_start(
                    E2[:], emb_d[b, 0:L * D].rearrange("(h p f) -> p h f",
                                                       p=128, f=64)
                )
            EL = xpool.tile([1, 1], f32, tag="el")
            nc.sync.dma_start(EL[:], emb_d[b, L * D:L * D + 1][None, :])

            X2F = xpool.tile([128, 2, 64], f32, tag="x2f")
            nc.vector.tensor_tensor(out=X2F[:], in0=E2[:], in1=TWP[:],
                                    op=ALU.mult)

            # x1 = sum(X2F) + EL*twl + tb  (tb added on host)
            CS = xpool.tile([128, 2], f32, tag="cs")
            nc.vector.tensor_reduce(out=CS[:, 0:1], in_=X2F[:, 0, :],
                                    op=ALU.add, axis=mybir.AxisListType.X)
            nc.vector.tensor_reduce(out=CS[:, 1:2], in_=X2F[:, 1, :],
                                    op=ALU.add, axis=mybir.AxisListType.X)
            CS1 = xpool.tile([128, 1], f32, tag="cs1")
            nc.vector.tensor_tensor(out=CS1[:], in0=CS[:, 0:1], in1=CS[:, 1:2],
                                    op=ALU.add)
            PX1 = ppx.tile([1, 1], f32, tag="px1")
            nc.tensor.matmul(PX1[:], CS1[:], ONES[:], start=True, stop=False,
                             skip_group_check=True)
            nc.tensor.matmul(PX1[:], EL[:], TWL[:], start=False, stop=True,
                             skip_group_check=True)
            X1S = xpool.tile([1, 1], f32, tag="x1s")
            nc.vector.tensor_scalar(out=X1S[:], in0=PX1[:], scalar1=1.0,
                                    scalar2=None, op0=ALU.mult)
            nc.sync.dma_start(x1_o[b], X1S[:])

            # transpose X -> X2T [64d, 256i] in psum
            PT = ppt.tile([64, 256], f32, tag="pt")
            nc.tensor.matmul(PT[:, 0:128], X2F[:, 0, :], IDENT[:],
                             is_transpose=True, start=True, stop=True,
                             skip_group_check=True)
            nc.tensor.matmul(PT[:, 128:256], X2F[:, 1, :], IDENT[:],
                             is_transpose=True, start=True, stop=True,
                             skip_group_check=True)
            return PT

        def emit_setup_b(PT):
            """XU tile: [X2T | X2T] on both partition halves."""
            XU = xpool.tile([128, 512], bf16, tag="xu")
            nc.vector.tensor_scalar(out=XU[0:64, 0:256], in0=PT[:],
                                    scalar1=1.0, scalar2=None, op0=ALU.mult)
            nc.sync.dma_start(XU[0:64, 256:512], XU[0:64, 0:256])
            nc.sync.dma_start(XU[64:128, :], XU[0:64, :])
            return XU

        def emit_setup_c(XU):
            """XSo/XSe shifted tiles (spread across scalar/gpsimd queues)."""
            XSo = xpool.tile([128, 512], bf16, tag="xso")
            XSe = xpool.tile([128, 512], bf16, tag="xse")
            # XSo: top shift 1, bottom shift 65
            nc.sync.dma_start(XSo[0:64, 0:511], XU[0:64, 1:512])
            nc.sync.dma_start(XSo[0:64, 511:512], XU[0:64, 255:256])
            nc.sync.dma_start(XSo[64:128, 0:447], XU[0:64, 65:512])
            nc.sync.dma_start(XSo[64:128, 447:512], XU[0:64, 0:65])
            # XSe: top shift 2, bottom shift 66
            nc.gpsimd.dma_start(XSe[0:64, 0:510], XU[0:64, 2:512])
            nc.gpsimd.dma_start(XSe[0:64, 510:512], XU[0:64, 254:256])
            nc.gpsimd.dma_start(XSe[64:128, 0:446], XU[0:64, 66:512])
            nc.gpsimd.dma_start(XSe[64:128, 446:512], XU[0:64, 0:66])
            return XSo, XSe

        def build_a2(tiles, it):
            XU, XSo, XSe = tiles
            A2 = apool.tile([128, 1024], bf16, tag="a2")
            # cols 0:256   = shift (4it+1 | +65)   from XSo offset 4it
            # cols 256:512 = shift (4it+3 | +67)   from XSo offset 4it+2
            # cols 512:768 = shift (4it+2 | +66)   from XSe offset 4it
            # cols 768:1024= shift (4it+4 | +68)   from XSe offset 4it+2
            c0 = 4 * it
            in0b = XU[:, 0:256].unsqueeze(1).broadcast_to([128, 2, 256])
            for (dst, src) in ((A2[:, 0:512], XSo), (A2[:, 512:1024], XSe)):
                sl = src[:, c0:c0 + 258]
                in1w = bass.AP(tensor=sl.tensor, offset=sl.offset,
                               ap=[list(sl.ap[0]), [2, 2], [1, 256]])
                nc.vector.tensor_tensor(
                    out=dst.rearrange("p (a b) -> p a b", a=2),
                    in0=in0b, in1=in1w, op=ALU.subtract)
            nc.vector.tensor_scalar(
                out=A2[:].bitcast(u16), in0=A2[:].bitcast(u16),
                scalar1=0x7FFF, scalar2=None, op0=ALU.bitwise_and)
            return A2

        def emit_p1(tiles, A2, it):
            XU, XSo, XSe = tiles
            c0 = 4 * it
            # P1a: T0 (rows 0:64 <- o=u1,u1p) T10 (rows 64:128 <- +64)
            # P1b: T8 (rows 0:64 <- o=u2+64,u2p+64) T2 (rows 64:128 <- u2,u2p)
            P1 = pp1.tile([128, 1024], f32, tag="p1")
            for (pc, tp, ar) in (
                (0, (0, 0), 0),        # T0
                (0, (64, 64), 64),     # T10
                (512, (64, 0), 64),    # T8  (array rows 64-127 -> psum 0-63)
                (512, (0, 64), 0),     # T2  (array rows 0-63 -> psum 64-127)
            ):
                rg, pr = ar, tp[1]
                XSx = XSo if pc == 0 else XSe
                ps = P1[pr:pr + 64, pc:pc + 512]
                nc.tensor.matmul(ps, WB[rg:rg + 64, :], A2[rg:rg + 64, pc:pc + 512],
                                 start=True, stop=False, tile_position=tp,
                                 skip_group_check=True)
                nc.tensor.matmul(ps, WA[rg:rg + 64, :],
                                 XU[rg:rg + 64, 0:512],
                                 start=False, stop=False, tile_position=tp,
                                 skip_group_check=True)
                nc.tensor.matmul(ps[:, 0:256], WA[rg:rg + 64, :],
                                 XSx[rg:rg + 64, c0:c0 + 256],
                                 start=False, stop=False, tile_position=tp,
                                 skip_group_check=True)
                nc.tensor.matmul(ps[:, 256:512], WA[rg:rg + 64, :],
                                 XSx[rg:rg + 64, c0 + 2:c0 + 258],
                                 start=False, stop=True, tile_position=tp,
                                 skip_group_check=True)
            H1 = hpool.tile([128, 1024], bf16, tag="h1")
            nc.scalar.activation(H1[:], P1[:], AF_MAIN, bias=B1S[:],
                                 scale=1.0, alpha=0.01)
            return H1

        p2state = [None]

        def emit_layer2(ACC, H1, it):
            # layer 2: pack 8 offsets x 16 outputs across 128 psum
            # partitions; P2 free dim is only 256 -> 4x cheaper lrelu2.
            # One 128-contraction MM per 32-partition strip: weight rows
            # 0:64 (H1 top offset) land in cols 0:16, rows 64:128 in
            # 16:32 -- avoids two concurrent drains on one strip.
            # Two consecutive iterations share one 2-bank P2F tile (cols
            # 0:256 / 256:512) so lrelu2 + reduce run once per pair.
            # it=14 and the o=128 iteration (15) stay unpaired so the host
            # can halve the double-counted o=128 block (ACC cols 7 and 8).
            if it == 15:
                pair, po, col = (15,), 0, 8
            elif it == 14:
                pair, po, col = (14,), 0, 7
            else:
                pair, po, col = (it - it % 2, it - it % 2 + 1), 256 * (it % 2), it // 2
            if po == 0:
                P2F = pp2.tile([128, 512], f32, tag="p2")
                p2state[0] = P2F
            else:
                P2F = p2state[0]
            for cb in range(4):
                hc = 256 * cb
                nc.tensor.matmul(P2F[32 * cb:32 * cb + 32, po:po + 256],
                                 W2P8[:, 32 * cb:32 * cb + 32],
                                 H1[:, hc:hc + 256],
                                 start=True, stop=True,
                                 tile_position=(0, 32 * cb),
                                 skip_group_check=True)
            if it != pair[-1]:
                return
            n = 256 * len(pair)
            # lrelu2 on ACT (bias free), column-sum over i on DVE
            H2 = jpool.tile([128, 512], bf16, tag="h2")
            nc.scalar.activation(H2[:, 0:n], P2F[:, 0:n], AF_MAIN,
                                 bias=B2S8[:], scale=1.0, alpha=0.01)
            nc.vector.tensor_reduce(out=ACC[:, col:col + 1], in_=H2[:, 0:n],
                                    op=ALU.add, axis=mybir.AxisListType.X)

        # ---- main loops: 16 iterations x 8 offsets per batch ----
        # offsets at iteration it: T0 stream: u1=4it+1, u1p=4it+3
        #                          T2/T8 stream: u2=4it+2, u2p=4it+4
        #                          +64 variants on the bottom halves
        # Software-pipelined so the PE queue never stalls: A2 is built one
        # iteration ahead (DVE overlaps P1[it]); layer2[it-1] (whose MMs wait
        # on act1[it-1], finished during P1[it]) is emitted after act1[it];
        # the NEXT batch's setup chain is emitted mid-loop so its DMAs and
        # transposes hide under the current batch's compute.
        PT0 = emit_setup_a(0)
        XU0 = emit_setup_b(PT0)
        tiles = (XU0,) + emit_setup_c(XU0)
        A2_cur = build_a2(tiles, 0)
        pipelined = None  # (b, ACC, H1, it) -- lags one iteration, across batches
        for b in range(N_BATCH):
            ACC = opool.tile([128, 9], f32, tag="acc")
            next_pt = next_xu = next_tiles = None
            for it in range(nit):
                A2 = A2_cur
                if it + 1 < nit:
                    A2_cur = build_a2(tiles, it + 1)
                H1 = emit_p1(tiles, A2, it)
                if pipelined is not None:
                    pb, pacc, ph1, pit = pipelined
                    emit_layer2(pacc, ph1, pit)
                    if pit == nit - 1:
                        nc.sync.dma_start(acc_o[pb], pacc[:])
                pipelined = (b, ACC, H1, it)
                if b + 1 < N_BATCH:
                    if it == 1:
                        next_pt = emit_setup_a(b + 1)
                    elif it == 3:
                        next_xu = emit_setup_b(next_pt)
                    elif it == 5:
                        next_tiles = (next_xu,) + emit_setup_c(next_xu)
                    elif it == nit - 1:
                        A2_cur = build_a2(next_tiles, 0)
            if next_tiles is not None:
                tiles = next_tiles
        pb, pacc, ph1, pit = pipelined
        emit_layer2(pacc, ph1, pit)
        nc.sync.dma_start(acc_o[pb], pacc[:])

    nc.compile()
    return nc


def _get_program():
    key = (N_DUALS, N_BATCH)
    if key not in _CACHE:
        _CACHE[key] = _build_program()
    return _CACHE[key]


def _get_runner():
    """Build (once) a cached jitted SPMD executable for the program."""
    key = ("runner", N_DUALS, N_BATCH, N_RUN_CORES)
    if key in _CACHE:
        return _CACHE[key]
    import jax
    import jax.numpy as jnp
    import numpy as _np
    import concourse.mybir as mybir
    from jax.sharding import Mesh, PartitionSpec
    from jax.experimental.shard_map import shard_map
    from concourse import bass2jax
    from concourse.bass2jax import _bass_exec_p, partition_id_tensor

    bass2jax.install_neuronx_cc_hook()
    nc = _get_program()
    n_cores = N_RUN_CORES

    partition_name = (nc.partition_id_tensor.name
                      if nc.partition_id_tensor else None)
    in_names, out_names, out_avals, zero_shapes = [], [], [], []
    for alloc in nc.m.functions[0].allocations:
        if not isinstance(alloc, mybir.MemoryLocationSet):
            continue
        name = alloc.memorylocations[0].name
        if alloc.kind == "ExternalInput":
            if name != partition_name:
                in_names.append(name)
        elif alloc.kind == "ExternalOutput":
            out_names.append(name)
            shape = tuple(alloc.tensor_shape)
            dtype = mybir.dt.np(alloc.dtype)
            out_avals.append(jax.core.ShapedArray(shape, dtype))
            zero_shapes.append((shape, dtype))
    n_params = len(in_names)
    n_outs = len(out_avals)
    all_in_names = list(in_names) + list(out_names)
    if partition_name is not None:
        all_in_names.append(partition_name)
    donate = tuple(range(n_params, n_params + n_outs))

    def _body(*args):
        operands = list(args)
        if partition_name is not None:
            operands.append(partition_id_tensor())
        outs = _bass_exec_p.bind(
            *operands, out_avals=tuple(out_avals), in_names=tuple(all_in_names),
            out_names=tuple(out_names), lowering_input_output_aliases=(),
            sim_require_finite=True, sim_require_nnan=True, nc=nc)
        return tuple(outs)

    devices = jax.devices()[:n_cores]
    mesh = Mesh(_np.asarray(devices), ("core",))
    in_specs = (PartitionSpec("core"),) * (n_params + n_outs)
    out_specs = (PartitionSpec("core"),) * len(out_names)
    sharded = jax.jit(
        shard_map(_body, mesh=mesh, in_specs=in_specs, out_specs=out_specs,
                  check_rep=False),
        donate_argnums=donate, keep_unused=True)

    def run(in_maps):
        concat_in = [
            np.concatenate([np.asarray(in_maps[c][nm]) for c in range(n_cores)],
                           axis=0)
            for nm in in_names
        ]
        concat_zeros = [np.zeros((n_cores * s[0], *s[1:]), d)
                        for (s, d) in zero_shapes]
        out_arrs = sharded(*concat_in, *concat_zeros)
        return [
            {nm: np.asarray(out_arrs[i]).reshape(n_cores, *out_avals[i].shape)[c]
             for i, nm in enumerate(out_names)}
            for c in range(n_cores)
        ]

    _CACHE[key] = run
    return run


def _make_in_maps(emb, tw, w1, b1, w2, b2):
    import ml_dtypes

    emb = np.asarray(emb, np.float32)
    tw = np.asarray(tw, np.float32)
    w1 = np.asarray(w1, np.float32)
    b1v = np.asarray(b1, np.float32)
    b2v = np.asarray(b2, np.float32)

    bfl = ml_dtypes.bfloat16
    w1bt = np.ascontiguousarray(w1[:, 64:].T).astype(bfl)          # [64, 64]
    w1at = np.ascontiguousarray(0.5 * w1[:, :64].T).astype(bfl)    # [64, 64]
    w2f = np.asarray(w2, np.float32)
    # W2P8 [128, 128]: strip cb's [128, 32] weight: contraction rows 0:64
    # (H1 top offset) output at local cols 0:16, rows 64:128 at 16:32.
    w2p8 = np.zeros((128, 128), np.float32)
    for cb in range(4):
        w2p8[0:64, 32 * cb:32 * cb + 16] = w2f.T
        w2p8[64:128, 32 * cb + 16:32 * cb + 32] = w2f.T
    w2p8 = w2p8.astype(bfl)
    b1s = np.concatenate([b1v, b1v]).reshape(128, 1).astype(np.float32)
    b2s8 = np.tile(b2v, 8).reshape(128, 1).astype(np.float32)
    twp = np.ascontiguousarray(tw[:-1].reshape(L, D)).astype(np.float32)
    twl = np.array([[tw[-1]]], np.float32)

    shared = {
        "w1bt": w1bt, "w1at": w1at, "w2p8": w2p8,
        "b1s": b1s, "b2s8": b2s8, "twp": twp, "twl": twl,
    }
    in_maps = []
    for c in range(N_CORES):
        m = dict(shared)
        m["emb4"] = np.ascontiguousarray(emb[c * B_PER_CORE:(c + 1) * B_PER_CORE])
        in_maps.append(m)
    return in_maps


def kernel(emb, tw, tb, w1, b1, w2, b2, w3, b3, scale):
    run = _get_runner()
    in_maps = _make_in_maps(emb, tw, w1, b1, w2, b2)
    core_results = run(in_maps[:N_RUN_CORES])

    w3v = np.asarray(w3, np.float32)[0]
    out = np.zeros(32, np.float32)
    for c in range(N_RUN_CORES):
        r = core_results[c]
        acc = r["acc_o"]            # [4, 128, 16]
        x1p = r["x1_o"][:, 0, 0]    # [4]
        for b in range(N_BATCH):
            m16 = acc[b].reshape(128, 9)
            R = m16.reshape(8, 16, 9).sum(axis=(0, 2))
            # o=128 (col 8, partitions 96:112) was double counted
            R -= 0.5 * m16[96:112, 8]
            out[c * B_PER_CORE + b] = (
                x1p[b] + float(tb[0])
                + float(scale[0]) * (R @ w3v + float(b3[0]) * NPAIRS)
            )
    return out



# revision 29
# speedup vs baseline: 1.0841x; 1.0219x over previous
"""Trainium2 Bass kernel for nn_EpiNN_aaindex (pairwise-MLP GNN reduction).

Math (per batch b):
  x1 = emb@tw + tb
  X[i,d] = emb[i*64+d] * tw[i*64+d]            (L=256, D=64)
  s_ij = MLP(concat[(x_i+x_j)/2, |x_i-x_j|])   (64->16->1, LeakyReLU 0.01)
  out_b = x1 + scale * sum_{i<j} s_ij

Strategy: 8 cores, 4 batches/core (data parallel over B=32).
Exact upper-triangle enumeration via cyclic offsets o=1..128:
pairs (i, (i+o) mod 256) for o=1..127 cover each unordered pair once;
o=128 covers each of its 128 pairs twice (weighted 0.5 on the host).

Layouts (per batch, SBUF, bf16):
  XU  [128, 512]: both partition halves = [X2T | X2T]  (X2T = X.T [64, 256])
  XSo [128, 512]: top = X2T shifted 1, bottom = shifted 65   (odd offsets)
  XSe [128, 512]: top = X2T shifted 2, bottom = shifted 66   (even offsets)
Unit u = offset pair (u, u+64); dual-unit d = units (2d+1, 2d+2).
A2 [128, 512] = |x_i - x_j| features for 4 offsets (2 per partition half).
P1 [128, 512] psum quadrants = pre1 for the 4 offsets (3 matmuls each:
  w1b@A, 0.5*w1a@X2T (u_i term), 0.5*w1a@X2T-shifted (u_j term)).
ACT Lrelu (+b1 bias) -> H1 bf16 -> 4 layer-2 matmuls -> P2 -> ACT Lrelu
(+b2) with accum_out giving the free-dim (i) sums for free.

Final combine on host: out = x1 + scale*(w3 . R + 32640*b3).
"""
import numpy as np

L, D = 256, 64
B_PER_CORE = 4
N_CORES = 8
NPAIRS = 32640  # 256*255/2

_CACHE = {}
import os as _os
N_DUALS = int(_os.environ.get("EPINN_DUALS", "32"))
N_BATCH = int(_os.environ.get("EPINN_BATCH", str(B_PER_CORE)))
N_RUN_CORES = int(_os.environ.get("EPINN_CORES", str(N_CORES)))
STAGE = int(_os.environ.get("EPINN_STAGE", "9"))
VAR = _os.environ.get("EPINN_VAR", "")


def _build_program():
    import concourse.bacc as bacc
    import concourse.bass as bass
    import concourse.mybir as mybir
    import concourse.tile as tile
    from contextlib import ExitStack

    f32 = mybir.dt.float32
    bf16 = mybir.dt.bfloat16
    u16 = mybir.dt.uint16
    AF = mybir.ActivationFunctionType
    ALU = mybir.AluOpType
    AF_MAIN = AF.Relu if _os.environ.get("EPINN_RELU") else AF.Lrelu

    nc = bacc.Bacc("TRN2", target_bir_lowering=False, debug=False,
                   num_devices=N_CORES)

    # ---- DRAM parameters (per core) ----
    emb_d = nc.declare_dram_parameter("emb4", [B_PER_CORE, L * D + 1], f32,
                                      isOutput=False)
    w1bt_d = nc.declare_dram_parameter("w1bt", [64, 64], bf16, isOutput=False)
    w1at_d = nc.declare_dram_parameter("w1at", [64, 64], bf16, isOutput=False)
    w2p8_d = nc.declare_dram_parameter("w2p8", [128, 128], bf16, isOutput=False)
    b1s_d = nc.declare_dram_parameter("b1s", [128, 1], f32, isOutput=False)
    b2s8_d = nc.declare_dram_parameter("b2s8", [128, 1], f32, isOutput=False)
    twp_d = nc.declare_dram_parameter("twp", [L, D], f32, isOutput=False)
    twl_d = nc.declare_dram_parameter("twl", [1, 1], f32, isOutput=False)

    acc_o = nc.declare_dram_parameter("acc_o", [B_PER_CORE, 128, 9], f32,
                                      isOutput=True)
    x1_o = nc.declare_dram_parameter("x1_o", [B_PER_CORE, 1, 1], f32,
                                     isOutput=True)

    with tile.TileContext(nc) as tc, ExitStack() as ctx:
        cpool = ctx.enter_context(tc.tile_pool(name="consts", bufs=1))
        xpool = ctx.enter_context(tc.tile_pool(name="xbufs", bufs=2))
        apool = ctx.enter_context(tc.tile_pool(name="abufs", bufs=3))
        hpool = ctx.enter_context(tc.tile_pool(name="hbufs", bufs=3))
        jpool = ctx.enter_context(tc.tile_pool(name="junk", bufs=2))
        opool = ctx.enter_context(tc.tile_pool(name="outs", bufs=2))
        pp1 = ctx.enter_context(tc.tile_pool(name="p1", bufs=2, space="PSUM"))
        pp2 = ctx.enter_context(tc.tile_pool(name="p2", bufs=2, space="PSUM"))
        ppt = ctx.enter_context(tc.tile_pool(name="pt", bufs=1, space="PSUM"))
        ppx = ctx.enter_context(tc.tile_pool(name="px", bufs=1, space="PSUM"))

        # ---- static weights / consts ----
        WB = cpool.tile([128, 64], bf16)
        WA = cpool.tile([128, 64], bf16)
        W2P8 = cpool.tile([128, 128], bf16)
        B1S = cpool.tile([128, 1], f32)
        B2S8 = cpool.tile([128, 1], f32)
        TWP = cpool.tile([128, 2, 64], f32)  # [128p, (half, d)]
        TWL = cpool.tile([1, 1], f32)
        IDENT = cpool.tile([128, 128], f32)
        ONES = cpool.tile([128, 1], f32)

        E2_0 = xpool.tile([128, 2, 64], f32, tag="e2")
        nc.sync.dma_start(
            E2_0[:], emb_d[0, 0:L * D].rearrange("(h p f) -> p h f", p=128, f=64)
        )
        nc.gpsimd.dma_start(TWP[:, 0, :], twp_d[0:128, :])
        nc.gpsimd.dma_start(TWP[:, 1, :], twp_d[128:256, :])
        nc.sync.dma_start(WB[0:64, :], w1bt_d[:])
        nc.sync.dma_start(WB[64:128, :], w1bt_d[:])
        nc.sync.dma_start(WA[0:64, :], w1at_d[:])
        nc.sync.dma_start(WA[64:128, :], w1at_d[:])
        nc.sync.dma_start(W2P8[:], w2p8_d[:])
        nc.scalar.dma_start(B1S[:], b1s_d[:])
        nc.scalar.dma_start(B2S8[:], b2s8_d[:])
        nc.scalar.dma_start(TWL[:], twl_d[:])
        nc.gpsimd.memset(IDENT[:], 0.0)
        nc.gpsimd.affine_select(
            out=IDENT[:], in_=IDENT[:], compare_op=ALU.not_equal, fill=1.0,
            base=0, pattern=[[-1, 128]], channel_multiplier=1,
        )
        nc.gpsimd.memset(ONES[:], 1.0)

        nit = N_DUALS // 2

        def emit_setup_a(b):
            """emb load, X = emb*tw, x1 scalar path, transpose to PT."""
            if b == 0:
                E2 = E2_0
            else:
                E2 = xpool.tile([128, 2, 64], f32, tag="e2")
                nc.sync.dma_start(
                    E2[:], emb_d[b, 0:L * D].rearrange("(h p f) -> p h f",
                                                       p=128, f=64)
                )
            EL = xpool.tile([1, 1], f32, tag="el")
            nc.sync.dma_start(EL[:], emb_d[b, L * D:L * D + 1][None, :])

            X2F = xpool.tile([128, 2, 64], f32, tag="x2f")
            nc.vector.tensor_tensor(out=X2F[:], in0=E2[:], in1=TWP[:],
                                    op=ALU.mult)

            # x1 = sum(X2F) + EL*twl + tb  (tb added on host)
            CS = xpool.tile([128, 2], f32, tag="cs")
            nc.vector.tensor_reduce(out=CS[:, 0:1], in_=X2F[:, 0, :],
                                    op=ALU.add, axis=mybir.AxisListType.X)
            nc.vector.tensor_reduce(out=CS[:, 1:2], in_=X2F[:, 1, :],
                                    op=ALU.add, axis=mybir.AxisListType.X)
            CS1 = xpool.tile([128, 1], f32, tag="cs1")
            nc.vector.tensor_tensor(out=CS1[:], in0=CS[:, 0:1], in1=CS[:, 1:2],
                                    op=ALU.add)
            PX1 = ppx.tile([1, 1], f32, tag="px1")
            nc.tensor.matmul(PX1[:], CS1[:], ONES[:], start=True, stop=False,
                             skip_group_check=True)
            nc.tensor.matmul(PX1[:], EL[:], TWL[:], start=False, stop=True,
                             skip_group_check=True)
            X1S = xpool.tile([1, 1], f32, tag="x1s")
            nc.vector.tensor_scalar(out=X1S[:], in0=PX1[:], scalar1=1.0,
                                    scalar2=None, op0=ALU.mult)
            nc.sync.dma_start(x1_o[b], X1S[:])

            # transpose X -> X2T [64d, 256i] in psum
            PT = ppt.tile([64, 256], f32, tag="pt")
            nc.tensor.matmul(PT[:, 0:128], X2F[:, 0, :], IDENT[:],
                             is_transpose=True, start=True, stop=True,
                             skip_group_check=True)
            nc.tensor.matmul(PT[:, 128:256], X2F[:, 1, :], IDENT[:],
                             is_transpose=True, start=True, stop=True,
                             skip_group_check=True)
            return PT

        def emit_setup_b(PT, first=False):
            """XU tile: [X2T | X2T] on both partition halves."""
            dup = nc.scalar if first else nc.sync
            XU = xpool.tile([128, 512], bf16, tag="xu")
            nc.vector.tensor_scalar(out=XU[0:64, 0:256], in0=PT[:],
                                    scalar1=1.0, scalar2=None, op0=ALU.mult)
            dup.dma_start(XU[0:64, 256:512], XU[0:64, 0:256])
            dup.dma_start(XU[64:128, :], XU[0:64, :])
            return XU

        def emit_setup_c(XU, first=False):
            """XSo/XSe shifted tiles on lightly-loaded DGE queues (the
            scalar queue is saturated by activations mid-loop, but free
            during the batch-0 prologue)."""
            so = nc.scalar if first else nc.sync
            XSo = xpool.tile([128, 512], bf16, tag="xso")
            XSe = xpool.tile([128, 512], bf16, tag="xse")
            # XSo: top shift 1, bottom shift 65
            so.dma_start(XSo[0:64, 0:511], XU[0:64, 1:512])
            so.dma_start(XSo[0:64, 511:512], XU[0:64, 255:256])
            so.dma_start(XSo[64:128, 0:447], XU[0:64, 65:512])
            so.dma_start(XSo[64:128, 447:512], XU[0:64, 0:65])
            # XSe: top shift 2, bottom shift 66
            nc.gpsimd.dma_start(XSe[0:64, 0:510], XU[0:64, 2:512])
            nc.gpsimd.dma_start(XSe[0:64, 510:512], XU[0:64, 254:256])
            nc.gpsimd.dma_start(XSe[64:128, 0:446], XU[0:64, 66:512])
            nc.gpsimd.dma_start(XSe[64:128, 446:512], XU[0:64, 0:66])
            return XSo, XSe

        def build_a2(tiles, it):
            XU, XSo, XSe = tiles
            A2 = apool.tile([128, 1024], bf16, tag="a2")
            # cols 0:256   = shift (4it+1 | +65)   from XSo offset 4it
            # cols 256:512 = shift (4it+3 | +67)   from XSo offset 4it+2
            # cols 512:768 = shift (4it+2 | +66)   from XSe offset 4it
            # cols 768:1024= shift (4it+4 | +68)   from XSe offset 4it+2
            c0 = 4 * it
            in0b = XU[:, 0:256].unsqueeze(1).broadcast_to([128, 2, 256])
            for (dst, src) in ((A2[:, 0:512], XSo), (A2[:, 512:1024], XSe)):
                sl = src[:, c0:c0 + 258]
                in1w = bass.AP(tensor=sl.tensor, offset=sl.offset,
                               ap=[list(sl.ap[0]), [2, 2], [1, 256]])
                nc.vector.tensor_tensor(
                    out=dst.rearrange("p (a b) -> p a b", a=2),
                    in0=in0b, in1=in1w, op=ALU.subtract)
            nc.vector.tensor_scalar(
                out=A2[:].bitcast(u16), in0=A2[:].bitcast(u16),
                scalar1=0x7FFF, scalar2=None, op0=ALU.bitwise_and)
            return A2

        def emit_p1(tiles, A2, it):
            XU, XSo, XSe = tiles
            c0 = 4 * it
            # P1a: T0 (rows 0:64 <- o=u1,u1p) T10 (rows 64:128 <- +64)
            # P1b: T8 (rows 0:64 <- o=u2+64,u2p+64) T2 (rows 64:128 <- u2,u2p)
            P1 = pp1.tile([128, 1024], f32, tag="p1")
            for (pc, tp, ar) in (
                (0, (0, 0), 0),        # T0
                (0, (64, 64), 64),     # T10
                (512, (64, 0), 64),    # T8  (array rows 64-127 -> psum 0-63)
                (512, (0, 64), 0),     # T2  (array rows 0-63 -> psum 64-127)
            ):
                rg, pr = ar, tp[1]
                XSx = XSo if pc == 0 else XSe
                ps = P1[pr:pr + 64, pc:pc + 512]
                nc.tensor.matmul(ps, WB[rg:rg + 64, :], A2[rg:rg + 64, pc:pc + 512],
                                 start=True, stop=False, tile_position=tp,
                                 skip_group_check=True)
                nc.tensor.matmul(ps, WA[rg:rg + 64, :],
                                 XU[rg:rg + 64, 0:512],
                                 start=False, stop=False, tile_position=tp,
                                 skip_group_check=True)
                nc.tensor.matmul(ps[:, 0:256], WA[rg:rg + 64, :],
                                 XSx[rg:rg + 64, c0:c0 + 256],
                                 start=False, stop=False, tile_position=tp,
                                 skip_group_check=True)
                nc.tensor.matmul(ps[:, 256:512], WA[rg:rg + 64, :],
                                 XSx[rg:rg + 64, c0 + 2:c0 + 258],
                                 start=False, stop=True, tile_position=tp,
                                 skip_group_check=True)
            H1 = hpool.tile([128, 1024], bf16, tag="h1")
            nc.scalar.activation(H1[:], P1[:], AF_MAIN, bias=B1S[:],
                                 scale=1.0, alpha=0.01)
            return H1

        p2state = [None]

        def emit_layer2(ACC, H1, it):
            # layer 2: pack 8 offsets x 16 outputs across 128 psum
            # partitions; P2 free dim is only 256 -> 4x cheaper lrelu2.
            # One 128-contraction MM per 32-partition strip: weight rows
            # 0:64 (H1 top offset) land in cols 0:16, rows 64:128 in
            # 16:32 -- avoids two concurrent drains on one strip.
            # Two consecutive iterations share one 2-bank P2F tile (cols
            # 0:256 / 256:512) so lrelu2 + reduce run once per pair.
            # it=14 and the o=128 iteration (15) stay unpaired so the host
            # can halve the double-counted o=128 block (ACC cols 7 and 8).
            if it == 15:
                pair, po, col = (15,), 0, 8
            elif it == 14:
                pair, po, col = (14,), 0, 7
            else:
                pair, po, col = (it - it % 2, it - it % 2 + 1), 256 * (it % 2), it // 2
            if po == 0:
                P2F = pp2.tile([128, 512], f32, tag="p2")
                p2state[0] = P2F
            else:
                P2F = p2state[0]
            for cb in range(4):
                hc = 256 * cb
                nc.tensor.matmul(P2F[32 * cb:32 * cb + 32, po:po + 256],
                                 W2P8[:, 32 * cb:32 * cb + 32],
                                 H1[:, hc:hc + 256],
                                 start=True, stop=True,
                                 tile_position=(0, 32 * cb),
                                 skip_group_check=True)
            if it != pair[-1]:
                return
            n = 256 * len(pair)
            # lrelu2 on ACT (bias free), column-sum over i on DVE
            H2 = jpool.tile([128, 512], bf16, tag="h2")
            nc.scalar.activation(H2[:, 0:n], P2F[:, 0:n], AF_MAIN,
                                 bias=B2S8[:], scale=1.0, alpha=0.01)
            nc.vector.tensor_reduce(out=ACC[:, col:col + 1], in_=H2[:, 0:n],
                                    op=ALU.add, axis=mybir.AxisListType.X)

        # ---- main loops: 16 iterations x 8 offsets per batch ----
        # offsets at iteration it: T0 stream: u1=4it+1, u1p=4it+3
        #                          T2/T8 stream: u2=4it+2, u2p=4it+4
        #                          +64 variants on the bottom halves
        # Software-pipelined so the PE queue never stalls: A2 is built one
        # iteration ahead (DVE overlaps P1[it]); layer2[it-1] (whose MMs wait
        # on act1[it-1], finished during P1[it]) is emitted after act1[it];
        # the NEXT batch's setup chain is emitted mid-loop so its DMAs and
        # transposes hide under the current batch's compute.
        PT0 = emit_setup_a(0)
        XU0 = emit_setup_b(PT0, first=True)
        tiles = (XU0,) + emit_setup_c(XU0, first=True)
        A2_cur = build_a2(tiles, 0)
        pipelined = None  # (b, ACC, H1, it) -- lags one iteration, across batches
        for b in range(N_BATCH):
            ACC = opool.tile([128, 9], f32, tag="acc")
            next_pt = next_xu = next_tiles = None
            for it in range(nit):
                A2 = A2_cur
                if it + 1 < nit:
                    A2_cur = build_a2(tiles, it + 1)
                H1 = emit_p1(tiles, A2, it)
                if pipelined is not None:
                    pb, pacc, ph1, pit = pipelined
                    emit_layer2(pacc, ph1, pit)
                    if pit == nit - 1:
                        nc.sync.dma_start(acc_o[pb], pacc[:])
                pipelined = (b, ACC, H1, it)
                if b + 1 < N_BATCH:
                    if it == 1:
                        next_pt = emit_setup_a(b + 1)
                    elif it == 3:
                        next_xu = emit_setup_b(next_pt)
                    elif it == 5:
                        next_tiles = (next_xu,) + emit_setup_c(next_xu)
                    elif it == nit - 1:
                        A2_cur = build_a2(next_tiles, 0)
            if next_tiles is not None:
                tiles = next_tiles
        pb, pacc, ph1, pit = pipelined
        emit_layer2(pacc, ph1, pit)
        nc.sync.dma_start(acc_o[pb], pacc[:])

    nc.compile()
    return nc


def _get_program():
    key = (N_DUALS, N_BATCH)
    if key not in _CACHE:
        _CACHE[key] = _build_program()
    return _CACHE[key]


def _get_runner():
    """Build (once) a cached jitted SPMD executable for the program."""
    key = ("runner", N_DUALS, N_BATCH, N_RUN_CORES)
    if key in _CACHE:
        return _CACHE[key]
    import jax
    import jax.numpy as jnp
    import numpy as _np
    import concourse.mybir as mybir
    from jax.sharding import Mesh, PartitionSpec
    from jax.experimental.shard_map import shard_map
    from concourse import bass2jax
    from concourse.bass2jax import _bass_exec_p, partition_id_tensor

    bass2jax.install_neuronx_cc_hook()
    nc = _get_program()
    n_cores = N_RUN_CORES

    partition_name = (nc.partition_id_tensor.name
                      if nc.partition_id_tensor else None)
    in_names, out_names, out_avals, zero_shapes = [], [], [], []
    for alloc in nc.m.functions[0].allocations:
        if not isinstance(alloc, mybir.MemoryLocationSet):
            continue
        name = alloc.memorylocations[0].name
        if alloc.kind == "ExternalInput":
            if name != partition_name:
                in_names.append(name)
        elif alloc.kind == "ExternalOutput":
            out_names.append(name)
            shape = tuple(alloc.tensor_shape)
            dtype = mybir.dt.np(alloc.dtype)
            out_avals.append(jax.core.ShapedArray(shape, dtype))
            zero_shapes.append((shape, dtype))
    n_params = len(in_names)
    n_outs = len(out_avals)
    all_in_names = list(in_names) + list(out_names)
    if partition_name is not None:
        all_in_names.append(partition_name)
    donate = tuple(range(n_params, n_params + n_outs))

    def _body(*args):
        operands = list(args)
        if partition_name is not None:
            operands.append(partition_id_tensor())
        outs = _bass_exec_p.bind(
            *operands, out_avals=tuple(out_avals), in_names=tuple(all_in_names),
            out_names=tuple(out_names), lowering_input_output_aliases=(),
            sim_require_finite=True, sim_require_nnan=True, nc=nc)
        return tuple(outs)

    devices = jax.devices()[:n_cores]
    mesh = Mesh(_np.asarray(devices), ("core",))
    in_specs = (PartitionSpec("core"),) * (n_params + n_outs)
    out_specs = (PartitionSpec("core"),) * len(out_names)
    sharded = jax.jit(
        shard_map(_body, mesh=mesh, in_specs=in_specs, out_specs=out_specs,
                  check_rep=False),
        donate_argnums=donate, keep_unused=True)

    def run(in_maps):
        concat_in = [
            np.concatenate([np.asarray(in_maps[c][nm]) for c in range(n_cores)],
                           axis=0)
            for nm in in_names
        ]
        concat_zeros = [np.zeros((n_cores * s[0], *s[1:]), d)
                        for (s, d) in zero_shapes]
        out_arrs = sharded(*concat_in, *concat_zeros)
        return [
            {nm: np.asarray(out_arrs[i]).reshape(n_cores, *out_avals[i].shape)[c]
             for i, nm in enumerate(out_names)}
            for c in range(n_cores)
        ]

    _CACHE[key] = run
    return run


def _make_in_maps(emb, tw, w1, b1, w2, b2):
    import ml_dtypes

    emb = np.asarray(emb, np.float32)
    tw = np.asarray(tw, np.float32)
    w1 = np.asarray(w1, np.float32)
    b1v = np.asarray(b1, np.float32)
    b2v = np.asarray(b2, np.float32)

    bfl = ml_dtypes.bfloat16
    w1bt = np.ascontiguousarray(w1[:, 64:].T).astype(bfl)          # [64, 64]
    w1at = np.ascontiguousarray(0.5 * w1[:, :64].T).astype(bfl)    # [64, 64]
    w2f = np.asarray(w2, np.float32)
    # W2P8 [128, 128]: strip cb's [128, 32] weight: contraction rows 0:64
    # (H1 top offset) output at local cols 0:16, rows 64:128 at 16:32.
    w2p8 = np.zeros((128, 128), np.float32)
    for cb in range(4):
        w2p8[0:64, 32 * cb:32 * cb + 16] = w2f.T
        w2p8[64:128, 32 * cb + 16:32 * cb + 32] = w2f.T
    w2p8 = w2p8.astype(bfl)
    b1s = np.concatenate([b1v, b1v]).reshape(128, 1).astype(np.float32)
    b2s8 = np.tile(b2v, 8).reshape(128, 1).astype(np.float32)
    twp = np.ascontiguousarray(tw[:-1].reshape(L, D)).astype(np.float32)
    twl = np.array([[tw[-1]]], np.float32)

    shared = {
        "w1bt": w1bt, "w1at": w1at, "w2p8": w2p8,
        "b1s": b1s, "b2s8": b2s8, "twp": twp, "twl": twl,
    }
    in_maps = []
    for c in range(N_CORES):
        m = dict(shared)
        m["emb4"] = np.ascontiguousarray(emb[c * B_PER_CORE:(c + 1) * B_PER_CORE])
        in_maps.append(m)
    return in_maps


def kernel(emb, tw, tb, w1, b1, w2, b2, w3, b3, scale):
    run = _get_runner()
    in_maps = _make_in_maps(emb, tw, w1, b1, w2, b2)
    core_results = run(in_maps[:N_RUN_CORES])

    w3v = np.asarray(w3, np.float32)[0]
    out = np.zeros(32, np.float32)
    for c in range(N_RUN_CORES):
        r = core_results[c]
        acc = r["acc_o"]            # [4, 128, 16]
        x1p = r["x1_o"][:, 0, 0]    # [4]
        for b in range(N_BATCH):
            m16 = acc[b].reshape(128, 9)
            R = m16.reshape(8, 16, 9).sum(axis=(0, 2))
            # o=128 (col 8, partitions 96:112) was double counted
            R -= 0.5 * m16[96:112, 8]
            out[c * B_PER_CORE + b] = (
                x1p[b] + float(tb[0])
                + float(scale[0]) * (R @ w3v + float(b3[0]) * NPAIRS)
            )
    return out



# revision 32
# speedup vs baseline: 1.1032x; 1.0176x over previous
"""Trainium2 Bass kernel for nn_EpiNN_aaindex (pairwise-MLP GNN reduction).

Math (per batch b):
  x1 = emb@tw + tb
  X[i,d] = emb[i*64+d] * tw[i*64+d]            (L=256, D=64)
  s_ij = MLP(concat[(x_i+x_j)/2, |x_i-x_j|])   (64->16->1, LeakyReLU 0.01)
  out_b = x1 + scale * sum_{i<j} s_ij

Strategy: 8 cores, 4 batches/core (data parallel over B=32).
Exact upper-triangle enumeration via cyclic offsets o=1..128:
pairs (i, (i+o) mod 256) for o=1..127 cover each unordered pair once;
o=128 covers each of its 128 pairs twice (weighted 0.5 on the host).

Layouts (per batch, SBUF, bf16):
  XU  [128, 512]: both partition halves = [X2T | X2T]  (X2T = X.T [64, 256])
  XSo [128, 512]: top = X2T shifted 1, bottom = shifted 65   (odd offsets)
  XSe [128, 512]: top = X2T shifted 2, bottom = shifted 66   (even offsets)
Unit u = offset pair (u, u+64); dual-unit d = units (2d+1, 2d+2).
A2 [128, 512] = |x_i - x_j| features for 4 offsets (2 per partition half).
P1 [128, 512] psum quadrants = pre1 for the 4 offsets (3 matmuls each:
  w1b@A, 0.5*w1a@X2T (u_i term), 0.5*w1a@X2T-shifted (u_j term)).
ACT Lrelu (+b1 bias) -> H1 bf16 -> 4 layer-2 matmuls -> P2 -> ACT Lrelu
(+b2) with accum_out giving the free-dim (i) sums for free.

Final combine on host: out = x1 + scale*(w3 . R + 32640*b3).
"""
import numpy as np

L, D = 256, 64
B_PER_CORE = 4
N_CORES = 8
NPAIRS = 32640  # 256*255/2

_CACHE = {}
import os as _os
N_DUALS = int(_os.environ.get("EPINN_DUALS", "32"))
N_BATCH = int(_os.environ.get("EPINN_BATCH", str(B_PER_CORE)))
N_RUN_CORES = int(_os.environ.get("EPINN_CORES", str(N_CORES)))
STAGE = int(_os.environ.get("EPINN_STAGE", "9"))
VAR = _os.environ.get("EPINN_VAR", "")


def _build_program():
    import concourse.bacc as bacc
    import concourse.bass as bass
    import concourse.mybir as mybir
    import concourse.tile as tile
    from contextlib import ExitStack

    f32 = mybir.dt.float32
    bf16 = mybir.dt.bfloat16
    u16 = mybir.dt.uint16
    AF = mybir.ActivationFunctionType
    ALU = mybir.AluOpType
    AF_MAIN = AF.Relu if _os.environ.get("EPINN_RELU") else AF.Lrelu

    nc = bacc.Bacc("TRN2", target_bir_lowering=False, debug=False,
                   num_devices=N_CORES)

    # ---- DRAM parameters (per core) ----
    emb_d = nc.declare_dram_parameter("emb4", [B_PER_CORE, L * D + 1], f32,
                                      isOutput=False)
    w1bt_d = nc.declare_dram_parameter("w1bt", [64, 64], bf16, isOutput=False)
    w1at_d = nc.declare_dram_parameter("w1at", [64, 64], bf16, isOutput=False)
    w2p8_d = nc.declare_dram_parameter("w2p8", [128, 128], bf16, isOutput=False)
    b1s_d = nc.declare_dram_parameter("b1s", [128, 1], f32, isOutput=False)
    b2s8_d = nc.declare_dram_parameter("b2s8", [128, 1], f32, isOutput=False)
    twp_d = nc.declare_dram_parameter("twp", [L, D], f32, isOutput=False)
    twl_d = nc.declare_dram_parameter("twl", [1, 1], f32, isOutput=False)

    acc_o = nc.declare_dram_parameter("acc_o", [B_PER_CORE, 128, 9], f32,
                                      isOutput=True)
    x1_o = nc.declare_dram_parameter("x1_o", [B_PER_CORE, 1, 1], f32,
                                     isOutput=True)

    with tile.TileContext(nc) as tc, ExitStack() as ctx:
        cpool = ctx.enter_context(tc.tile_pool(name="consts", bufs=1))
        xpool = ctx.enter_context(tc.tile_pool(name="xbufs", bufs=2))
        apool = ctx.enter_context(tc.tile_pool(name="abufs", bufs=3))
        hpool = ctx.enter_context(tc.tile_pool(name="hbufs", bufs=3))
        jpool = ctx.enter_context(tc.tile_pool(name="junk", bufs=2))
        opool = ctx.enter_context(tc.tile_pool(name="outs", bufs=2))
        pp1 = ctx.enter_context(tc.tile_pool(name="p1", bufs=2, space="PSUM"))
        pp2 = ctx.enter_context(tc.tile_pool(name="p2", bufs=2, space="PSUM"))
        ppt = ctx.enter_context(tc.tile_pool(name="pt", bufs=1, space="PSUM"))
        ppx = ctx.enter_context(tc.tile_pool(name="px", bufs=1, space="PSUM"))

        # ---- static weights / consts ----
        WB = cpool.tile([128, 64], bf16)
        WA = cpool.tile([128, 64], bf16)
        W2P8 = cpool.tile([128, 128], bf16)
        B1S = cpool.tile([128, 1], f32)
        B2S8 = cpool.tile([128, 1], f32)
        TWP = cpool.tile([128, 2, 64], f32)  # [128p, (half, d)]
        TWL = cpool.tile([1, 1], f32)
        IDENT = cpool.tile([128, 128], f32)
        ONES = cpool.tile([128, 1], f32)

        E2_0 = xpool.tile([128, 2, 64], f32, tag="e2")
        nc.sync.dma_start(
            E2_0[:], emb_d[0, 0:L * D].rearrange("(h p f) -> p h f", p=128, f=64)
        )
        nc.gpsimd.dma_start(TWP[:, 0, :], twp_d[0:128, :])
        nc.gpsimd.dma_start(TWP[:, 1, :], twp_d[128:256, :])
        nc.sync.dma_start(WB[0:64, :], w1bt_d[:])
        nc.sync.dma_start(WB[64:128, :], w1bt_d[:])
        nc.sync.dma_start(WA[0:64, :], w1at_d[:])
        nc.sync.dma_start(WA[64:128, :], w1at_d[:])
        nc.sync.dma_start(W2P8[:], w2p8_d[:])
        nc.scalar.dma_start(B1S[:], b1s_d[:])
        nc.scalar.dma_start(B2S8[:], b2s8_d[:])
        nc.scalar.dma_start(TWL[:], twl_d[:])
        nc.gpsimd.memset(IDENT[:], 0.0)
        nc.gpsimd.affine_select(
            out=IDENT[:], in_=IDENT[:], compare_op=ALU.not_equal, fill=1.0,
            base=0, pattern=[[-1, 128]], channel_multiplier=1,
        )
        nc.gpsimd.memset(ONES[:], 1.0)

        nit = N_DUALS // 2

        def emit_setup_a(b):
            """emb load, X = emb*tw, x1 scalar path, transpose to PT."""
            if b == 0:
                E2 = E2_0
            else:
                E2 = xpool.tile([128, 2, 64], f32, tag="e2")
                nc.sync.dma_start(
                    E2[:], emb_d[b, 0:L * D].rearrange("(h p f) -> p h f",
                                                       p=128, f=64)
                )
            EL = xpool.tile([1, 1], f32, tag="el")
            nc.sync.dma_start(EL[:], emb_d[b, L * D:L * D + 1][None, :])

            X2F = xpool.tile([128, 2, 64], f32, tag="x2f")
            nc.vector.tensor_tensor(out=X2F[:], in0=E2[:], in1=TWP[:],
                                    op=ALU.mult)

            # x1 = sum(X2F) + EL*twl + tb  (tb added on host)
            CS = xpool.tile([128, 2], f32, tag="cs")
            nc.vector.tensor_reduce(out=CS[:, 0:1], in_=X2F[:, 0, :],
                                    op=ALU.add, axis=mybir.AxisListType.X)
            nc.vector.tensor_reduce(out=CS[:, 1:2], in_=X2F[:, 1, :],
                                    op=ALU.add, axis=mybir.AxisListType.X)
            CS1 = xpool.tile([128, 1], f32, tag="cs1")
            nc.vector.tensor_tensor(out=CS1[:], in0=CS[:, 0:1], in1=CS[:, 1:2],
                                    op=ALU.add)
            PX1 = ppx.tile([1, 1], f32, tag="px1")
            nc.tensor.matmul(PX1[:], CS1[:], ONES[:], start=True, stop=False,
                             skip_group_check=True)
            nc.tensor.matmul(PX1[:], EL[:], TWL[:], start=False, stop=True,
                             skip_group_check=True)
            X1S = xpool.tile([1, 1], f32, tag="x1s")
            nc.vector.tensor_scalar(out=X1S[:], in0=PX1[:], scalar1=1.0,
                                    scalar2=None, op0=ALU.mult)
            nc.sync.dma_start(x1_o[b], X1S[:])

            # transpose X -> X2T [64d, 256i] in psum
            PT = ppt.tile([64, 256], f32, tag="pt")
            nc.tensor.matmul(PT[:, 0:128], X2F[:, 0, :], IDENT[:],
                             is_transpose=True, start=True, stop=True,
                             skip_group_check=True)
            nc.tensor.matmul(PT[:, 128:256], X2F[:, 1, :], IDENT[:],
                             is_transpose=True, start=True, stop=True,
                             skip_group_check=True)
            return PT

        def emit_setup_b(PT, first=False):
            """XU tile: [X2T | X2T] on both partition halves."""
            dup = nc.scalar if first else nc.sync
            XU = xpool.tile([128, 512], bf16, tag="xu")
            nc.vector.tensor_scalar(out=XU[0:64, 0:256], in0=PT[:],
                                    scalar1=1.0, scalar2=None, op0=ALU.mult)
            dup.dma_start(XU[0:64, 256:512], XU[0:64, 0:256])
            dup.dma_start(XU[64:128, :], XU[0:64, :])
            return XU

        def emit_setup_c(XU, first=False):
            """XSo/XSe shifted tiles on lightly-loaded DGE queues (the
            scalar queue is saturated by activations mid-loop, but free
            during the batch-0 prologue)."""
            so = nc.scalar if first else nc.sync
            XSo = xpool.tile([128, 512], bf16, tag="xso")
            XSe = xpool.tile([128, 512], bf16, tag="xse")
            # XSo: top shift 1, bottom shift 65
            so.dma_start(XSo[0:64, 0:511], XU[0:64, 1:512])
            so.dma_start(XSo[0:64, 511:512], XU[0:64, 255:256])
            so.dma_start(XSo[64:128, 0:447], XU[0:64, 65:512])
            so.dma_start(XSo[64:128, 447:512], XU[0:64, 0:65])
            # XSe: top shift 2, bottom shift 66
            nc.gpsimd.dma_start(XSe[0:64, 0:510], XU[0:64, 2:512])
            nc.gpsimd.dma_start(XSe[0:64, 510:512], XU[0:64, 254:256])
            nc.gpsimd.dma_start(XSe[64:128, 0:446], XU[0:64, 66:512])
            nc.gpsimd.dma_start(XSe[64:128, 446:512], XU[0:64, 0:66])
            return XSo, XSe

        def build_a2(tiles, pr):
            """|x_i - x_j| features for a PAIR of iterations (2pr, 2pr+1):
            one [128, 2048] tile, cols 0:1024 = odd offsets 8pr+{1,3,5,7},
            cols 1024:2048 = even offsets 8pr+{2,4,6,8} (each block 256 wide,
            partition halves carry the +64 offsets)."""
            XU, XSo, XSe = tiles
            A2 = apool.tile([128, 2048], bf16, tag="a2")
            c0 = 8 * pr
            in0b = XU[:, 0:256].unsqueeze(1).broadcast_to([128, 4, 256])
            for (dst, src) in ((A2[:, 0:1024], XSo), (A2[:, 1024:2048], XSe)):
                sl = src[:, c0:c0 + 262]
                in1w = bass.AP(tensor=sl.tensor, offset=sl.offset,
                               ap=[list(sl.ap[0]), [2, 4], [1, 256]])
                nc.vector.tensor_tensor(
                    out=dst.rearrange("p (a b) -> p a b", a=4),
                    in0=in0b, in1=in1w, op=ALU.subtract)
            nc.vector.tensor_scalar(
                out=A2[:].bitcast(u16), in0=A2[:].bitcast(u16),
                scalar1=0x7FFF, scalar2=None, op0=ALU.bitwise_and)
            return A2

        def emit_p1(tiles, A2, it):
            XU, XSo, XSe = tiles
            c0 = 4 * it
            # P1a: T0 (rows 0:64 <- o=u1,u1p) T10 (rows 64:128 <- +64)
            # P1b: T8 (rows 0:64 <- o=u2+64,u2p+64) T2 (rows 64:128 <- u2,u2p)
            P1 = pp1.tile([128, 1024], f32, tag="p1")
            half = 512 * (it % 2)
            for (pc, tp, ar) in (
                (0, (0, 0), 0),        # T0
                (0, (64, 64), 64),     # T10
                (512, (64, 0), 64),    # T8  (array rows 64-127 -> psum 0-63)
                (512, (0, 64), 0),     # T2  (array rows 0-63 -> psum 64-127)
            ):
                rg, pr = ar, tp[1]
                XSx = XSo if pc == 0 else XSe
                ac = 2 * pc + half
                ps = P1[pr:pr + 64, pc:pc + 512]
                nc.tensor.matmul(ps, WB[rg:rg + 64, :], A2[rg:rg + 64, ac:ac + 512],
                                 start=True, stop=False, tile_position=tp,
                                 skip_group_check=True)
                nc.tensor.matmul(ps, WA[rg:rg + 64, :],
                                 XU[rg:rg + 64, 0:512],
                                 start=False, stop=False, tile_position=tp,
                                 skip_group_check=True)
                nc.tensor.matmul(ps[:, 0:256], WA[rg:rg + 64, :],
                                 XSx[rg:rg + 64, c0:c0 + 256],
                                 start=False, stop=False, tile_position=tp,
                                 skip_group_check=True)
                nc.tensor.matmul(ps[:, 256:512], WA[rg:rg + 64, :],
                                 XSx[rg:rg + 64, c0 + 2:c0 + 258],
                                 start=False, stop=True, tile_position=tp,
                                 skip_group_check=True)
            H1 = hpool.tile([128, 1024], bf16, tag="h1")
            nc.scalar.activation(H1[:], P1[:], AF_MAIN, bias=B1S[:],
                                 scale=1.0, alpha=0.01)
            return H1

        p2state = [None]

        def emit_layer2(ACC, H1, it):
            # layer 2: pack 8 offsets x 16 outputs across 128 psum
            # partitions; P2 free dim is only 256 -> 4x cheaper lrelu2.
            # One 128-contraction MM per 32-partition strip: weight rows
            # 0:64 (H1 top offset) land in cols 0:16, rows 64:128 in
            # 16:32 -- avoids two concurrent drains on one strip.
            # Two consecutive iterations share one 2-bank P2F tile (cols
            # 0:256 / 256:512) so lrelu2 + reduce run once per pair.
            # it=14 and the o=128 iteration (15) stay unpaired so the host
            # can halve the double-counted o=128 block (ACC cols 7 and 8).
            if it == 15:
                pair, po, col = (15,), 0, 8
            elif it == 14:
                pair, po, col = (14,), 0, 7
            else:
                pair, po, col = (it - it % 2, it - it % 2 + 1), 256 * (it % 2), it // 2
            if po == 0:
                P2F = pp2.tile([128, 512], f32, tag="p2")
                p2state[0] = P2F
            else:
                P2F = p2state[0]
            for cb in range(4):
                hc = 256 * cb
                nc.tensor.matmul(P2F[32 * cb:32 * cb + 32, po:po + 256],
                                 W2P8[:, 32 * cb:32 * cb + 32],
                                 H1[:, hc:hc + 256],
                                 start=True, stop=True,
                                 tile_position=(0, 32 * cb),
                                 skip_group_check=True)
            if it != pair[-1]:
                return
            n = 256 * len(pair)
            # lrelu2 on ACT (bias free), column-sum over i on DVE
            H2 = jpool.tile([128, 512], bf16, tag="h2")
            nc.scalar.activation(H2[:, 0:n], P2F[:, 0:n], AF_MAIN,
                                 bias=B2S8[:], scale=1.0, alpha=0.01)
            nc.vector.tensor_reduce(out=ACC[:, col:col + 1], in_=H2[:, 0:n],
                                    op=ALU.add, axis=mybir.AxisListType.X)

        # ---- main loops: 16 iterations x 8 offsets per batch ----
        # offsets at iteration it: T0 stream: u1=4it+1, u1p=4it+3
        #                          T2/T8 stream: u2=4it+2, u2p=4it+4
        #                          +64 variants on the bottom halves
        # Software-pipelined so the PE queue never stalls: A2 is built one
        # iteration ahead (DVE overlaps P1[it]); layer2[it-1] (whose MMs wait
        # on act1[it-1], finished during P1[it]) is emitted after act1[it];
        # the NEXT batch's setup chain is emitted mid-loop so its DMAs and
        # transposes hide under the current batch's compute.
        PT0 = emit_setup_a(0)
        XU0 = emit_setup_b(PT0, first=True)
        tiles = (XU0,) + emit_setup_c(XU0, first=True)
        A2_cur = build_a2(tiles, 0)
        pipelined = None  # (b, ACC, H1, it) -- lags one iteration, across batches
        for b in range(N_BATCH):
            ACC = opool.tile([128, 9], f32, tag="acc")
            next_pt = next_xu = next_tiles = None
            for it in range(nit):
                A2 = A2_cur
                if it % 2 == 0 and it + 2 < nit:
                    A2_next = build_a2(tiles, it // 2 + 1)
                elif it % 2 == 1:
                    A2_cur = A2_next
                H1 = emit_p1(tiles, A2, it)
                if pipelined is not None:
                    pb, pacc, ph1, pit = pipelined
                    emit_layer2(pacc, ph1, pit)
                    if pit == nit - 1:
                        nc.sync.dma_start(acc_o[pb], pacc[:])
                pipelined = (b, ACC, H1, it)
                if b + 1 < N_BATCH:
                    if it == 1:
                        next_pt = emit_setup_a(b + 1)
                    elif it == 3:
                        next_xu = emit_setup_b(next_pt)
                    elif it == 5:
                        next_tiles = (next_xu,) + emit_setup_c(next_xu)
                    elif it == nit - 1:
                        A2_cur = build_a2(next_tiles, 0)  # pair 0 of next batch
            if next_tiles is not None:
                tiles = next_tiles
        pb, pacc, ph1, pit = pipelined
        emit_layer2(pacc, ph1, pit)
        nc.sync.dma_start(acc_o[pb], pacc[:])

    nc.compile()
    return nc


def _get_program():
    key = (N_DUALS, N_BATCH)
    if key not in _CACHE:
        _CACHE[key] = _build_program()
    return _CACHE[key]


def _get_runner():
    """Build (once) a cached jitted SPMD executable for the program."""
    key = ("runner", N_DUALS, N_BATCH, N_RUN_CORES)
    if key in _CACHE:
        return _CACHE[key]
    import jax
    import jax.numpy as jnp
    import numpy as _np
    import concourse.mybir as mybir
    from jax.sharding import Mesh, PartitionSpec
    from jax.experimental.shard_map import shard_map
    from concourse import bass2jax
    from concourse.bass2jax import _bass_exec_p, partition_id_tensor

    bass2jax.install_neuronx_cc_hook()
    nc = _get_program()
    n_cores = N_RUN_CORES

    partition_name = (nc.partition_id_tensor.name
                      if nc.partition_id_tensor else None)
    in_names, out_names, out_avals, zero_shapes = [], [], [], []
    for alloc in nc.m.functions[0].allocations:
        if not isinstance(alloc, mybir.MemoryLocationSet):
            continue
        name = alloc.memorylocations[0].name
        if alloc.kind == "ExternalInput":
            if name != partition_name:
                in_names.append(name)
        elif alloc.kind == "ExternalOutput":
            out_names.append(name)
            shape = tuple(alloc.tensor_shape)
            dtype = mybir.dt.np(alloc.dtype)
            out_avals.append(jax.core.ShapedArray(shape, dtype))
            zero_shapes.append((shape, dtype))
    n_params = len(in_names)
    n_outs = len(out_avals)
    all_in_names = list(in_names) + list(out_names)
    if partition_name is not None:
        all_in_names.append(partition_name)
    donate = tuple(range(n_params, n_params + n_outs))

    def _body(*args):
        operands = list(args)
        if partition_name is not None:
            operands.append(partition_id_tensor())
        outs = _bass_exec_p.bind(
            *operands, out_avals=tuple(out_avals), in_names=tuple(all_in_names),
            out_names=tuple(out_names), lowering_input_output_aliases=(),
            sim_require_finite=True, sim_require_nnan=True, nc=nc)
        return tuple(outs)

    devices = jax.devices()[:n_cores]
    mesh = Mesh(_np.asarray(devices), ("core",))
    in_specs = (PartitionSpec("core"),) * (n_params + n_outs)
    out_specs = (PartitionSpec("core"),) * len(out_names)
    sharded = jax.jit(
        shard_map(_body, mesh=mesh, in_specs=in_specs, out_specs=out_specs,
                  check_rep=False),
        donate_argnums=donate, keep_unused=True)

    def run(in_maps):
        concat_in = [
            np.concatenate([np.asarray(in_maps[c][nm]) for c in range(n_cores)],
                           axis=0)
            for nm in in_names
        ]
        concat_zeros = [np.zeros((n_cores * s[0], *s[1:]), d)
                        for (s, d) in zero_shapes]
        out_arrs = sharded(*concat_in, *concat_zeros)
        return [
            {nm: np.asarray(out_arrs[i]).reshape(n_cores, *out_avals[i].shape)[c]
             for i, nm in enumerate(out_names)}
            for c in range(n_cores)
        ]

    _CACHE[key] = run
    return run


def _make_in_maps(emb, tw, w1, b1, w2, b2):
    import ml_dtypes

    emb = np.asarray(emb, np.float32)
    tw = np.asarray(tw, np.float32)
    w1 = np.asarray(w1, np.float32)
    b1v = np.asarray(b1, np.float32)
    b2v = np.asarray(b2, np.float32)

    bfl = ml_dtypes.bfloat16
    w1bt = np.ascontiguousarray(w1[:, 64:].T).astype(bfl)          # [64, 64]
    w1at = np.ascontiguousarray(0.5 * w1[:, :64].T).astype(bfl)    # [64, 64]
    w2f = np.asarray(w2, np.float32)
    # W2P8 [128, 128]: strip cb's [128, 32] weight: contraction rows 0:64
    # (H1 top offset) output at local cols 0:16, rows 64:128 at 16:32.
    w2p8 = np.zeros((128, 128), np.float32)
    for cb in range(4):
        w2p8[0:64, 32 * cb:32 * cb + 16] = w2f.T
        w2p8[64:128, 32 * cb + 16:32 * cb + 32] = w2f.T
    w2p8 = w2p8.astype(bfl)
    b1s = np.concatenate([b1v, b1v]).reshape(128, 1).astype(np.float32)
    b2s8 = np.tile(b2v, 8).reshape(128, 1).astype(np.float32)
    twp = np.ascontiguousarray(tw[:-1].reshape(L, D)).astype(np.float32)
    twl = np.array([[tw[-1]]], np.float32)

    shared = {
        "w1bt": w1bt, "w1at": w1at, "w2p8": w2p8,
        "b1s": b1s, "b2s8": b2s8, "twp": twp, "twl": twl,
    }
    in_maps = []
    for c in range(N_CORES):
        m = dict(shared)
        m["emb4"] = np.ascontiguousarray(emb[c * B_PER_CORE:(c + 1) * B_PER_CORE])
        in_maps.append(m)
    return in_maps


def kernel(emb, tw, tb, w1, b1, w2, b2, w3, b3, scale):
    run = _get_runner()
    in_maps = _make_in_maps(emb, tw, w1, b1, w2, b2)
    core_results = run(in_maps[:N_RUN_CORES])

    w3v = np.asarray(w3, np.float32)[0]
    out = np.zeros(32, np.float32)
    for c in range(N_RUN_CORES):
        r = core_results[c]
        acc = r["acc_o"]            # [4, 128, 16]
        x1p = r["x1_o"][:, 0, 0]    # [4]
        for b in range(N_BATCH):
            m16 = acc[b].reshape(128, 9)
            R = m16.reshape(8, 16, 9).sum(axis=(0, 2))
            # o=128 (col 8, partitions 96:112) was double counted
            R -= 0.5 * m16[96:112, 8]
            out[c * B_PER_CORE + b] = (
                x1p[b] + float(tb[0])
                + float(scale[0]) * (R @ w3v + float(b3[0]) * NPAIRS)
            )
    return out

